# revision 1
# baseline (speedup 1.0000x reference)
"""Trainium2 Bass kernel for nn_GRU_15461882266204 (minGRU with causal conv gate).

Math (reference):
  w0 = x @ w_w.T ; z0 = x @ wz_w.T ; th = x @ wh_w.T          (S,H)
  z  = sigmoid(causal_conv4(z0, conv_w, segment-masked))
  a  = (1-z) * (1-start) ; b = z * th
  h_t = a_t * h_{t-1} + b_t                                    (scan over S)
  out = (h * silu(w0)) @ wo_w.T                                (S,D)

Strategy: sequence-parallel over 8 NeuronCores (1024 positions each, all 5632
channels per core). Projections as fp32r matmuls with D on the contraction
partitions (x pre-transposed on host). Conv + gating elementwise on DVE with
host-precomputed boundary masks (fully data-driven, SPMD-identical program).
Scan via the hardware tensor_tensor_scan instruction (channels on partitions,
time on the free axis). Cross-core scan carry: each core computes a local scan
from 0 plus the in-chunk cumprod A; a 360KB AllGather of (A_end, h_end)
summaries lets every core compute the carry chain redundantly and fix up
h_true = h_local + A * carry. Down-projection in bf16; output is
sequence-sharded so the host just concatenates (no all-reduce).
"""
import sys

sys.path.insert(0, "/opt/trn_rl_repo")

import numpy as np

import concourse.bacc as bacc
import concourse.mybir as mybir
import concourse.tile as tile
from concourse.bass_utils import run_bass_kernel_spmd

try:
    import ml_dtypes

    BF16 = np.dtype(ml_dtypes.bfloat16)
except ImportError:  # pragma: no cover
    BF16 = None

F32 = mybir.dt.float32
F32R = mybir.dt.float32r
MBF16 = mybir.dt.bfloat16
AL = mybir.AluOpType
ACTF = mybir.ActivationFunctionType

P = 128
CONV = 4
# matmul input dtype: bf16 enables the fast weight load path (LDWEIGHTS fully
# hidden behind the matmul stream); fp32r is ~1.3x slower but more accurate.
MM_BF16 = True
# 3 history columns are needed for the conv taps; we pad to 4 (one dead
# leading column) because fp32r matmuls require an even moving free-dim.
HIST = 4


def _ntiles(total, maxn=512):
    """Chop `total` into pieces of at most maxn: [(offset, size), ...]."""
    out = []
    o = 0
    while o < total:
        n = min(maxn, total - o)
        out.append((o, n))
        o += n
    return out


def build_gru_kernel(D, H, SC, NC, phases="ABCD"):
    """Build the SPMD per-core program. SC = sequence chunk per core."""
    KT = D // P    # contraction k-tiles
    MT = H // P    # hidden m-tiles
    SCH = SC + HIST
    z_nt = _ntiles(SCH)        # n-tiles for z_pre (includes 3 history cols)
    s_nt = _ntiles(SC)         # n-tiles for ht / w0
    MPT = SC // P              # output row tiles (s on partitions)
    NPT_D = _ntiles(D)         # output col tiles of 512
    # down-proj blocking: a m'-tiles x b n'-tiles concurrently, a*b <= 8 psum banks
    a_blk = min(MPT, 4)
    b_blk = min(len(NPT_D), 2)

    nc = bacc.Bacc(None, target_bir_lowering=False, debug=False)

    mmdt = MBF16 if MM_BF16 else F32
    xt_in = nc.declare_dram_parameter("xt", [P, KT, SCH], mmdt, isOutput=False)
    wz_in = nc.declare_dram_parameter("wz", [MT, P, KT, P], mmdt, isOutput=False)
    wh_in = nc.declare_dram_parameter("wh", [MT, P, KT, P], mmdt, isOutput=False)
    w_in = nc.declare_dram_parameter("w", [MT, P, KT, P], mmdt, isOutput=False)
    wo_in = nc.declare_dram_parameter("wo", [MT, P, D], MBF16, isOutput=False)
    cw_in = nc.declare_dram_parameter("cw", [MT, P, CONV], F32, isOutput=False)
    u_in = nc.declare_dram_parameter("u", [P, SC + 2], F32, isOutput=False)
    sel_in = nc.declare_dram_parameter("sel", [P, NC], F32, isOutput=False)
    out_d = nc.declare_dram_parameter("out", [SC, D], F32, isOutput=True)
    dbg_out = None
    if phases != "ABCD":
        # debug dumps: h_loc, A (phase A) / mycarry / g
        dbg_out = nc.declare_dram_parameter(
            "dbg", [3, MT, P, SC], F32, isOutput=True
        )

    with tile.TileContext(nc) as tc:
        with (
            tc.tile_pool(name="const", bufs=1) as cpool,
            tc.tile_pool(name="wts", bufs=2) as wpool,
            tc.tile_pool(name="work", bufs=2) as wk,
            tc.tile_pool(name="psum", bufs=8, space="PSUM") as pp,
            tc.tile_pool(name="dram", bufs=1, space="DRAM") as dp,
        ):
            # ---- resident tiles -------------------------------------------------
            sbdt = MBF16 if MM_BF16 else F32R
            xt_sb = cpool.tile([P, KT, SCH], sbdt, tag="xt")
            nc.sync.dma_start(xt_sb[:], xt_in[:])
            u_sb = cpool.tile([P, SC + 2], F32, tag="u")
            nc.sync.dma_start(u_sb[:], u_in[:])
            sel_sb = cpool.tile([P, NC], F32, tag="sel")
            nc.sync.dma_start(sel_sb[:], sel_in[:])
            ones = cpool.tile([P, SC], F32, tag="ones")
            nc.any.memset(ones[:], 1.0)
            summA = cpool.tile([P, MT], F32, tag="summA")
            summH = cpool.tile([P, MT], F32, tag="summH")

            # internal DRAM bounce buffers
            hl_d = dp.tile([MT, P, SC], F32)
            A_d = dp.tile([MT, P, SC], F32)
            g_d = dp.tile([MT, P, SC], MBF16)
            summ_d = dp.tile([P, 2 * MT], F32)
            gath_d = dp.tile([NC, P, 2 * MT], F32, addr_space="Shared")

            # ---- phase A: z/ht matmuls, conv, gating, local scans ---------------
            scopeA = nc.named_scope("phaseA"); scopeA.__enter__()
            for m in range(MT):
                cw_sb = wk.tile([P, CONV], F32, tag="cw")
                nc.sync.dma_start(cw_sb[:], cw_in[m])
                wz_sb = wpool.tile([P, KT, P], sbdt, tag="wz")
                nc.sync.dma_start(wz_sb[:], wz_in[m])
                wh_sb = wpool.tile([P, KT, P], sbdt, tag="wh")
                nc.sync.dma_start(wh_sb[:], wh_in[m])

                # z_pre = wz_m.T @ x over SC+3 cols (3 history cols included)
                z_pre = wk.tile([P, SCH], F32, tag="zpre")
                for (no, nn) in z_nt:
                    ps = pp.tile([P, 512], F32, tag="ps")
                    for k in range(KT):
                        nc.tensor.matmul(
                            ps[:, :nn],
                            wz_sb[:, k, :],
                            xt_sb[:, k, no : no + nn],
                            start=(k == 0),
                            stop=(k == KT - 1),
                        )
                    nc.scalar.copy(z_pre[:, no : no + nn], ps[:, :nn])

                # ht matmuls (positions [0, SC) = cols [3, SCH))
                ps_h = []
                for (no, nn) in s_nt:
                    ps = pp.tile([P, 512], F32, tag="ps")
                    for k in range(KT):
                        nc.tensor.matmul(
                            ps[:, :nn],
                            wh_sb[:, k, :],
                            xt_sb[:, k, HIST + no : HIST + no + nn],
                            start=(k == 0),
                            stop=(k == KT - 1),
                        )
                    ps_h.append((no, nn, ps))

                # masked shifted taps: yk(t) = u(t) * y{k-1}(t-1), y0 = z_pre
                # y1 covers t in [-2, SC), y2 [-1, SC), y3 [0, SC)
                y1 = wk.tile([P, SC + 2], F32, tag="y1")
                nc.vector.tensor_tensor(
                    y1[:], u_sb[:, : SC + 2], z_pre[:, HIST - 3 : HIST - 3 + SC + 2],
                    AL.mult,
                )
                y2 = wk.tile([P, SC + 1], F32, tag="y2")
                nc.vector.tensor_tensor(
                    y2[:], u_sb[:, 1 : SC + 2], y1[:, : SC + 1], AL.mult
                )
                y3 = wk.tile([P, SC], F32, tag="y3")
                nc.vector.tensor_tensor(
                    y3[:], u_sb[:, 2 : SC + 2], y2[:, :SC], AL.mult
                )
                # conv accumulation: acc = z*cw3 + y1*cw2 + y2*cw1 + y3*cw0
                acc = wk.tile([P, SC], F32, tag="acc")
                nc.vector.tensor_scalar(
                    acc[:], z_pre[:, HIST:SCH], cw_sb[:, 3:4], None, AL.mult
                )
                nc.vector.scalar_tensor_tensor(
                    acc[:], y1[:, 2 : SC + 2], cw_sb[:, 2:3], acc[:], AL.mult, AL.add
                )
                nc.vector.scalar_tensor_tensor(
                    acc[:], y2[:, 1 : SC + 1], cw_sb[:, 1:2], acc[:], AL.mult, AL.add
                )
                nc.vector.scalar_tensor_tensor(
                    acc[:], y3[:, :SC], cw_sb[:, 0:1], acc[:], AL.mult, AL.add
                )

                z_t = wk.tile([P, SC], F32, tag="y3")
                nc.scalar.activation(z_t[:], acc[:], ACTF.Sigmoid)
                na = wk.tile([P, SC], F32, tag="acc")
                nc.scalar.activation(na[:], acc[:], ACTF.Sigmoid, scale=-1.0)

                a_t = wk.tile([P, SC], F32, tag="a")
                nc.vector.tensor_tensor(a_t[:], na[:], u_sb[:, 2 : SC + 2], AL.mult)
                b_t = wk.tile([P, SC], F32, tag="b")
                for (no, nn, ps) in ps_h:
                    nc.vector.tensor_tensor(
                        b_t[:, no : no + nn], z_t[:, no : no + nn], ps[:, :nn], AL.mult
                    )

                h_loc = wk.tile([P, SC], F32, tag="hl")
                nc.vector.tensor_tensor_scan(
                    h_loc[:], a_t[:], b_t[:], 0.0, AL.mult, AL.add
                )
                A_t = wk.tile([P, SC], F32, tag="A")
                nc.vector.tensor_tensor_scan(
                    A_t[:], a_t[:], ones[:], 1.0, AL.mult, AL.mult
                )

                nc.scalar.copy(summA[:, m : m + 1], A_t[:, SC - 1 : SC])
                nc.scalar.copy(summH[:, m : m + 1], h_loc[:, SC - 1 : SC])
                nc.sync.dma_start(hl_d[m], h_loc[:])
                nc.sync.dma_start(A_d[m], A_t[:])
                if dbg_out is not None:
                    nc.sync.dma_start(dbg_out[0, m], h_loc[:])
                    nc.sync.dma_start(dbg_out[1, m], A_t[:])

            scopeA.__exit__(None, None, None)
            # ---- phase B: carry exchange ---------------------------------------
            mycarry = None
            if "B" in phases:
                nc.sync.dma_start(summ_d[:, 0:MT], summA[:])
                nc.sync.dma_start(summ_d[:, MT : 2 * MT], summH[:])
                nc.gpsimd.collective_compute(
                    "AllGather",
                    AL.bypass,
                    replica_groups=[list(range(NC))],
                    ins=[summ_d.opt()],
                    outs=[gath_d.opt()],
                )
                gsum = []
                for r in range(NC):
                    gs = cpool.tile([P, 2 * MT], F32, tag=f"gsum{r}", name=f"gsum{r}")
                    nc.sync.dma_start(gs[:], gath_d[r])
                    gsum.append(gs)
                state = cpool.tile([P, MT], F32, tag="cstate")
                tmp_c = cpool.tile([P, MT], F32, tag="ctmp")
                mycarry = cpool.tile([P, MT], F32, tag="mycarry")
                nc.any.memset(state[:], 0.0)
                nc.any.memset(mycarry[:], 0.0)
                for r in range(NC):
                    if r > 0:
                        # select carry entering rank r if this core is rank r
                        nc.vector.scalar_tensor_tensor(
                            mycarry[:], state[:], sel_sb[:, r : r + 1], mycarry[:],
                            AL.mult, AL.add,
                        )
                    if r < NC - 1:
                        nc.vector.tensor_tensor(
                            tmp_c[:], state[:], gsum[r][:, 0:MT], AL.mult
                        )
                        nc.vector.tensor_tensor(
                            state[:], tmp_c[:], gsum[r][:, MT : 2 * MT], AL.add
                        )
                if dbg_out is not None:
                    carry_dump = wk.tile([P, MT], F32, tag="cw")
                    nc.vector.tensor_copy(carry_dump[:], mycarry[:])
                    nc.sync.dma_start(dbg_out[2, 0, :, 0:MT], carry_dump[:])

            # ---- phase C: w0, silu, carry fixup, g -----------------------------
            if "C" in phases:
                scopeC = nc.named_scope("phaseC"); scopeC.__enter__()
                for m in range(MT):
                    w_sb = wpool.tile([P, KT, P], sbdt, tag="wz", name="w_sb")
                    nc.sync.dma_start(w_sb[:], w_in[m])
                    silu_t = wk.tile([P, SC], F32, tag="b")
                    for (no, nn) in s_nt:
                        ps = pp.tile([P, 512], F32, tag="ps", name="ps_w0")
                        for k in range(KT):
                            nc.tensor.matmul(
                                ps[:, :nn],
                                w_sb[:, k, :],
                                xt_sb[:, k, HIST + no : HIST + no + nn],
                                start=(k == 0),
                                stop=(k == KT - 1),
                            )
                        nc.scalar.activation(
                            silu_t[:, no : no + nn], ps[:, :nn], ACTF.Silu
                        )

                    hl_rd = wk.tile([P, SC], F32, tag="hl")
                    nc.sync.dma_start(hl_rd[:], hl_d[m])
                    A_rd = wk.tile([P, SC], F32, tag="A")
                    nc.sync.dma_start(A_rd[:], A_d[m])
                    h_true = wk.tile([P, SC], F32, tag="a")
                    nc.vector.scalar_tensor_tensor(
                        h_true[:], A_rd[:], mycarry[:, m : m + 1], hl_rd[:],
                        AL.mult, AL.add,
                    )
                    g_t = wk.tile([P, SC], MBF16, tag="g")
                    nc.vector.tensor_tensor(g_t[:], h_true[:], silu_t[:], AL.mult)
                    nc.sync.dma_start(g_d[m], g_t[:])
                    if dbg_out is not None and "D" not in phases:
                        if "S" in phases:
                            nc.sync.dma_start(dbg_out[2, m], silu_t[:])
                        else:
                            nc.sync.dma_start(dbg_out[2, m], h_true[:])

                scopeC.__exit__(None, None, None)

            # ---- phase D: down-projection out = g.T @ woT ----------------------
            if "D" in phases:
                scopeD = nc.named_scope("phaseD"); scopeD.__enter__()
                mp_all = list(range(MPT))
                for mbi in range(0, MPT, a_blk):
                    mps = mp_all[mbi : mbi + a_blk]
                    for nbi in range(0, len(NPT_D), b_blk):
                        nps = NPT_D[nbi : nbi + b_blk]
                        nb_off = nps[0][0]
                        nb_len = sum(nn for (_, nn) in nps)
                        ps_o = [
                            pp.tile([P, 512], F32, tag="ps", name=f"pso{i}")
                            for i in range(len(mps) * len(nps))
                        ]
                        for m in range(MT):
                            g_rd = wk.tile([P, P * len(mps)], MBF16, tag="y2")
                            nc.sync.dma_start(
                                g_rd[:], g_d[m][:, mbi * P : (mbi + len(mps)) * P]
                            )
                            wo_rd = wk.tile([P, nb_len], MBF16, tag="y1")
                            nc.sync.dma_start(
                                wo_rd[:], wo_in[m][:, nb_off : nb_off + nb_len]
                            )
                            for i_m in range(len(mps)):
                                for i_n, (no, nn) in enumerate(nps):
                                    nc.tensor.matmul(
                                        ps_o[i_m * len(nps) + i_n][:, :nn],
                                        g_rd[:, i_m * P : (i_m + 1) * P],
                                        wo_rd[:, no - nb_off : no - nb_off + nn],
                                        start=(m == 0),
                                        stop=(m == MT - 1),
                                    )
                        for i_m, mp in enumerate(mps):
                            o_sb = wk.tile([P, nb_len], F32, tag="zpre")
                            for i_n, (no, nn) in enumerate(nps):
                                nc.scalar.copy(
                                    o_sb[:, no - nb_off : no - nb_off + nn],
                                    ps_o[i_m * len(nps) + i_n][:, :nn],
                                )
                            nc.sync.dma_start(
                                out_d[mp * P : (mp + 1) * P, nb_off : nb_off + nb_len],
                                o_sb[:],
                            )
                scopeD.__exit__(None, None, None)
    nc.compile()
    return nc


def _prep_inputs(x, cu_seqlens, w_w, wz_w, wh_w, wo_w, conv_w, NC):
    """Host-side sharding + layout prep. Returns in_maps list."""
    S, D = x.shape[1], x.shape[2]
    H = w_w.shape[0]
    SC = S // NC
    KT, MT = D // P, H // P

    xT = np.ascontiguousarray(x[0].T.astype(np.float32))  # (D, S)
    xt_full = np.zeros((D, S + HIST), np.float32)
    xt_full[:, HIST:] = xT

    start = np.zeros(S, np.float32)
    for v in np.asarray(cu_seqlens[:-1]):
        v = int(v)
        if 0 <= v < S:
            start[v] = 1.0
    u = 1.0 - start
    u_full = np.ones(S + 2, np.float32)
    u_full[2:] = u  # index t+2 <-> position t

    mmnp = BF16 if MM_BF16 else np.float32

    def wprep(wm):  # (H, D) -> (MT, P, KT, P) with [m,p,k,j] = w[m*P+j, k*P+p]
        return np.ascontiguousarray(
            wm.astype(np.float32).reshape(MT, P, KT, P).transpose(0, 3, 2, 1)
        ).astype(mmnp)

    wz_t, wh_t, w_t = wprep(wz_w), wprep(wh_w), wprep(w_w)
    wo_t = np.ascontiguousarray(wo_w.T.astype(np.float32).reshape(MT, P, D)).astype(
        BF16
    )
    cw_t = np.ascontiguousarray(conv_w.astype(np.float32).reshape(MT, P, CONV))

    in_maps = []
    for c in range(NC):
        s0 = c * SC
        xt_c = np.ascontiguousarray(
            xt_full[:, s0 : s0 + SC + HIST]
            .reshape(KT, P, SC + HIST)
            .transpose(1, 0, 2)
        ).astype(mmnp)
        u_c = np.ascontiguousarray(
            np.broadcast_to(u_full[s0 : s0 + SC + 2], (P, SC + 2))
        )
        sel_c = np.zeros((P, NC), np.float32)
        sel_c[:, c] = 1.0
        in_maps.append(
            {
                "xt": xt_c,
                "wz": wz_t,
                "wh": wh_t,
                "w": w_t,
                "wo": wo_t,
                "cw": cw_t,
                "u": u_c,
                "sel": sel_c,
            }
        )
    return in_maps


_NC_CACHE = {}


def run_gru(x, cu_seqlens, w_w, wz_w, wh_w, wo_w, conv_w, NC=8, trace=False,
            phases="ABCD"):
    S, D = x.shape[1], x.shape[2]
    H = w_w.shape[0]
    SC = S // NC
    key = (D, H, SC, NC, phases)
    if key not in _NC_CACHE:
        _NC_CACHE[key] = build_gru_kernel(D, H, SC, NC, phases)
    nc = _NC_CACHE[key]
    in_maps = _prep_inputs(x, cu_seqlens, w_w, wz_w, wh_w, wo_w, conv_w, NC)
    res = run_bass_kernel_spmd(nc, in_maps, list(range(NC)), trace=trace)
    out = np.concatenate([res.results[c]["out"] for c in range(NC)], axis=0)
    return out.reshape(1, S, D).astype(np.float32), res


def kernel(**inputs):
    out, _ = run_gru(
        inputs["x"],
        inputs["cu_seqlens"],
        inputs["w_w"],
        inputs["wz_w"],
        inputs["wh_w"],
        inputs["wo_w"],
        inputs["conv_w"],
        NC=8,
    )
    return out



# revision 2
# speedup vs baseline: 1.0002x; 1.0002x over previous
"""Trainium2 Bass kernel for nn_GRU_15461882266204 (minGRU with causal conv gate).

Math (reference):
  w0 = x @ w_w.T ; z0 = x @ wz_w.T ; th = x @ wh_w.T          (S,H)
  z  = sigmoid(causal_conv4(z0, conv_w, segment-masked))
  a  = (1-z) * (1-start) ; b = z * th
  h_t = a_t * h_{t-1} + b_t                                    (scan over S)
  out = (h * silu(w0)) @ wo_w.T                                (S,D)

Strategy: sequence-parallel over 8 NeuronCores (1024 positions each, all 5632
channels per core). Phases:
  A1: per hidden m-tile: z/ht projections (bf16 or fp8-DoubleRow for z),
      causal conv + gates on DVE with host-precomputed boundary masks and
      host-precomputed 3-column z_pre history (kills all narrow matmuls),
      hardware tensor_tensor_scan for h_loc and the cumprod A. h_loc/A go to
      DRAM in bf16 (one merged DMA); chunk-end summaries stay in SBUF.
  B:  360KB AllGather of (A_end, h_end) summaries; every core redundantly
      computes the carry chain. Runs concurrently with A2.
  A2: w0 projection + silu, kept resident in SBUF (bf16).
  D:  fused carry fixup + down-projection: per output block, per m-tile:
      g = (h_loc + A*carry) * silu on DVE feeding bf16 matmuls accumulating
      over all 44 m-tiles in PSUM. Output is sequence-sharded; host concats.
"""
import sys

sys.path.insert(0, "/opt/trn_rl_repo")

import numpy as np

import concourse.bacc as bacc
import concourse.mybir as mybir
import concourse.tile as tile
from concourse.bass_utils import run_bass_kernel_spmd

import ml_dtypes

BF16 = np.dtype(ml_dtypes.bfloat16)
FP8 = np.dtype(ml_dtypes.float8_e4m3)

F32 = mybir.dt.float32
MBF16 = mybir.dt.bfloat16
MFP8 = mybir.dt.float8e4
AL = mybir.AluOpType
ACTF = mybir.ActivationFunctionType
PERF_DR = mybir.MatmulPerfMode.DoubleRow

P = 128
CONV = 4
# fp8 (e4m3, DoubleRow) for the z projection: its error is damped by the
# sigmoid; x scaled by 16, wz by 64 on host, descaled at PSUM drain.
FP8_Z = False
XSCALE = 16.0
WSCALE = 64.0
DESCALE = 1.0 / (XSCALE * WSCALE)


def build_gru_kernel(D, H, SC, NC, fp8_z=FP8_Z):
    KT = D // P          # contraction k-tiles (16)
    K2 = KT // 2         # fp8 DoubleRow k-tiles (8)
    MT = H // P          # hidden m-tiles (44)
    SCH = SC + 3         # z_pre cols incl 3 history cols
    MPT = SC // P        # seq row-tiles (8)

    nc = bacc.Bacc(None, target_bir_lowering=False, debug=False)

    xt_in = nc.declare_dram_parameter("xt", [P, KT, SC], MBF16, isOutput=False)
    wz_in = (
        nc.declare_dram_parameter("wz8", [MT, P, K2, 2, P], MFP8, isOutput=False)
        if fp8_z
        else nc.declare_dram_parameter("wz", [MT, P, KT, P], MBF16, isOutput=False)
    )
    if fp8_z:
        xt8_in = nc.declare_dram_parameter(
            "xt8", [P, K2, 2, SC], MFP8, isOutput=False
        )
    wh_in = nc.declare_dram_parameter("wh", [MT, P, KT, P], MBF16, isOutput=False)
    w_in = nc.declare_dram_parameter("w", [MT, P, KT, P], MBF16, isOutput=False)
    wo_in = nc.declare_dram_parameter("wo", [MT, P, D], MBF16, isOutput=False)
    # czh[m]: cols 0-3 conv_w taps, 4-6 host z_pre history, 7 pad
    czh_in = nc.declare_dram_parameter("czh", [MT, P, 8], F32, isOutput=False)
    u_in = nc.declare_dram_parameter("u", [P, SC + 2], F32, isOutput=False)
    sel_in = nc.declare_dram_parameter("sel", [P, NC], F32, isOutput=False)
    out_d = nc.declare_dram_parameter("out", [SC, D], F32, isOutput=True)

    with tile.TileContext(nc) as tc:
        with (
            tc.tile_pool(name="const", bufs=1) as cpool,
            tc.tile_pool(name="wts", bufs=2) as wpool,
            tc.tile_pool(name="work", bufs=2) as wk,
            tc.tile_pool(name="psum", bufs=8, space="PSUM") as pp,
            tc.tile_pool(name="dram", bufs=1, space="DRAM") as dp,
        ):
            # ---- resident tiles ------------------------------------------------
            xt_sb = cpool.tile([P, KT, SC], MBF16, tag="xt")
            nc.sync.dma_start(xt_sb[:], xt_in[:])
            if fp8_z:
                xt8_sb = cpool.tile([P, K2, 2, SC], MFP8, tag="xt8")
                nc.sync.dma_start(xt8_sb[:], xt8_in[:])
            u_sb = cpool.tile([P, SC + 2], F32, tag="u")
            nc.sync.dma_start(u_sb[:], u_in[:])
            sel_sb = cpool.tile([P, NC], F32, tag="sel")
            nc.sync.dma_start(sel_sb[:], sel_in[:])
            summA = cpool.tile([P, MT], F32, tag="summA")
            summH = cpool.tile([P, MT], F32, tag="summH")
            silu_sb = cpool.tile([P, MT, SC], MBF16, tag="silu")

            # internal DRAM bounce buffers
            hlA_d = dp.tile([MT, P, 2, SC], MBF16)  # [:,0,:]=A  [:,1,:]=h_loc
            summ_d = dp.tile([P, 2 * MT], F32)
            gath_d = dp.tile([NC, P, 2 * MT], F32, addr_space="Shared")

            # ---- phase A1: z/ht matmuls, conv, gating, local scans -------------
            scopeA = nc.named_scope("phaseA1"); scopeA.__enter__()
            for m in range(MT):
                czh = wk.tile([P, 8], F32, tag="czh")
                nc.sync.dma_start(czh[:], czh_in[m])
                if fp8_z:
                    wz_sb = wpool.tile([P, K2, 2, P], MFP8, tag="wz8")
                else:
                    wz_sb = wpool.tile([P, KT, P], MBF16, tag="wz")
                nc.sync.dma_start(wz_sb[:], wz_in[m])
                wh_sb = wpool.tile([P, KT, P], MBF16, tag="wh")
                nc.sync.dma_start(wh_sb[:], wh_in[m])

                # z_pre: cols 0-2 = host history, cols 3.. = matmul
                zp = wk.tile([P, SCH], F32, tag="zpre")
                nc.scalar.copy(zp[:, 0:3], czh[:, 4:7])
                for half in (0, 1):
                    ps = pp.tile([P, 512], F32, tag="ps")
                    if fp8_z:
                        for k2 in range(K2):
                            nc.tensor.matmul(
                                ps[:],
                                wz_sb[:, k2],
                                xt8_sb[:, k2, :, half * 512 : half * 512 + 512],
                                start=(k2 == 0),
                                stop=(k2 == K2 - 1),
                                perf_mode=PERF_DR,
                            )
                        nc.scalar.activation(
                            zp[:, 3 + half * 512 : 3 + half * 512 + 512],
                            ps[:],
                            ACTF.Copy,
                            scale=DESCALE,
                        )
                    else:
                        for k in range(KT):
                            nc.tensor.matmul(
                                ps[:],
                                wz_sb[:, k],
                                xt_sb[:, k, half * 512 : half * 512 + 512],
                                start=(k == 0),
                                stop=(k == KT - 1),
                            )
                        nc.scalar.copy(
                            zp[:, 3 + half * 512 : 3 + half * 512 + 512], ps[:]
                        )

                ps_h = []
                for half in (0, 1):
                    ps = pp.tile([P, 512], F32, tag="ps")
                    for k in range(KT):
                        nc.tensor.matmul(
                            ps[:],
                            wh_sb[:, k],
                            xt_sb[:, k, half * 512 : half * 512 + 512],
                            start=(k == 0),
                            stop=(k == KT - 1),
                        )
                    ps_h.append(ps)

                # conv: yk(t) = u(t)*y{k-1}(t-1), folded in-place into one tile
                y = wk.tile([P, SC + 2], F32, tag="y")
                nc.vector.tensor_tensor(
                    y[:], u_sb[:, : SC + 2], zp[:, : SC + 2], AL.mult
                )
                acc = wk.tile([P, SC], F32, tag="acc")
                nc.vector.tensor_scalar(
                    acc[:], zp[:, 3:SCH], czh[:, 3:4], None, AL.mult
                )
                nc.vector.scalar_tensor_tensor(
                    acc[:], y[:, 2 : SC + 2], czh[:, 2:3], acc[:], AL.mult, AL.add
                )
                nc.vector.tensor_tensor(
                    y[:, : SC + 1], u_sb[:, 1 : SC + 2], y[:, : SC + 1], AL.mult
                )
                nc.vector.scalar_tensor_tensor(
                    acc[:], y[:, 1 : SC + 1], czh[:, 1:2], acc[:], AL.mult, AL.add
                )
                nc.vector.tensor_tensor(
                    y[:, :SC], u_sb[:, 2 : SC + 2], y[:, :SC], AL.mult
                )
                nc.vector.scalar_tensor_tensor(
                    acc[:], y[:, :SC], czh[:, 0:1], acc[:], AL.mult, AL.add
                )

                zt = wk.tile([P, SC], F32, tag="zt")
                nc.scalar.activation(zt[:], acc[:], ACTF.Sigmoid)
                na = wk.tile([P, SC + 2], F32, tag="y", name="na")
                nc.scalar.activation(na[:, :SC], acc[:], ACTF.Sigmoid, scale=-1.0)
                # a = (1-z)*u, in place
                nc.vector.tensor_tensor(
                    na[:, :SC], na[:, :SC], u_sb[:, 2 : SC + 2], AL.mult
                )
                # b = z*th, in place over zt
                for half, ps in zip((0, 1), ps_h):
                    nc.vector.tensor_tensor(
                        zt[:, half * 512 : half * 512 + 512],
                        zt[:, half * 512 : half * 512 + 512],
                        ps[:],
                        AL.mult,
                    )

                hlA = wk.tile([P, 2, SC], MBF16, tag="hlA")
                nc.vector.tensor_tensor_scan(
                    hlA[:, 1, :], na[:, :SC], zt[:], 0.0, AL.mult, AL.add
                )
                # A-scan: u==1 wherever a!=0, so a*u*A == a*A
                nc.vector.tensor_tensor_scan(
                    hlA[:, 0, :], na[:, :SC], u_sb[:, 2 : SC + 2], 1.0,
                    AL.mult, AL.mult,
                )
                nc.scalar.copy(summA[:, m : m + 1], hlA[:, 0, SC - 1 : SC])
                nc.scalar.copy(summH[:, m : m + 1], hlA[:, 1, SC - 1 : SC])
                nc.sync.dma_start(hlA_d[m], hlA[:])
            scopeA.__exit__(None, None, None)

            # ---- phase B: carry exchange (overlaps phase A2) -------------------
            nc.sync.dma_start(summ_d[:, 0:MT], summA[:])
            nc.sync.dma_start(summ_d[:, MT : 2 * MT], summH[:])
            nc.gpsimd.collective_compute(
                "AllGather",
                AL.bypass,
                replica_groups=[list(range(NC))],
                ins=[summ_d.opt()],
                outs=[gath_d.opt()],
            )
            gsum = []
            for r in range(NC):
                gs = cpool.tile([P, 2 * MT], F32, tag=f"gsum{r}", name=f"gsum{r}")
                nc.sync.dma_start(gs[:], gath_d[r])
                gsum.append(gs)
            state = cpool.tile([P, MT], F32, tag="cstate")
            tmp_c = cpool.tile([P, MT], F32, tag="ctmp")
            mycarry = cpool.tile([P, MT], F32, tag="mycarry")
            nc.any.memset(state[:], 0.0)
            nc.any.memset(mycarry[:], 0.0)
            for r in range(NC):
                if r > 0:
                    nc.vector.scalar_tensor_tensor(
                        mycarry[:], state[:], sel_sb[:, r : r + 1], mycarry[:],
                        AL.mult, AL.add,
                    )
                if r < NC - 1:
                    nc.vector.tensor_tensor(
                        tmp_c[:], state[:], gsum[r][:, 0:MT], AL.mult
                    )
                    nc.vector.tensor_tensor(
                        state[:], tmp_c[:], gsum[r][:, MT : 2 * MT], AL.add
                    )

            # ---- phase A2: w0 projection + silu, resident in SBUF --------------
            scopeC = nc.named_scope("phaseA2"); scopeC.__enter__()
            for m in range(MT):
                w_sb = wpool.tile([P, KT, P], MBF16, tag="wh", name="w_sb")
                nc.sync.dma_start(w_sb[:], w_in[m])
                for half in (0, 1):
                    ps = pp.tile([P, 512], F32, tag="ps")
                    for k in range(KT):
                        nc.tensor.matmul(
                            ps[:],
                            w_sb[:, k],
                            xt_sb[:, k, half * 512 : half * 512 + 512],
                            start=(k == 0),
                            stop=(k == KT - 1),
                        )
                    nc.scalar.activation(
                        silu_sb[:, m, half * 512 : half * 512 + 512],
                        ps[:],
                        ACTF.Silu,
                    )
            scopeC.__exit__(None, None, None)

            # ---- phase D: fused carry fixup + down-projection ------------------
            scopeD = nc.named_scope("phaseD"); scopeD.__enter__()
            for mb in (0, 1):            # seq halves: row-tiles [mb*4, mb*4+4)
                for nbb in (0, 1):       # out-col halves: cols [nbb*1024, +1024)
                    ps_o = [
                        pp.tile([P, 512], F32, tag="ps", name=f"pso{mb}{nbb}{i}")
                        for i in range(8)
                    ]
                    for m in range(MT):
                        hlA_rd = wk.tile([P, 2, 512], MBF16, tag="hlard")
                        nc.sync.dma_start(
                            hlA_rd[:], hlA_d[m][:, :, mb * 512 : mb * 512 + 512]
                        )
                        wo_rd = wk.tile([P, 1024], MBF16, tag="word")
                        nc.sync.dma_start(
                            wo_rd[:], wo_in[m][:, nbb * 1024 : nbb * 1024 + 1024]
                        )
                        t1 = wk.tile([P, 512], MBF16, tag="t1")
                        nc.vector.scalar_tensor_tensor(
                            t1[:], hlA_rd[:, 0, :], mycarry[:, m : m + 1],
                            hlA_rd[:, 1, :], AL.mult, AL.add,
                        )
                        g = wk.tile([P, 512], MBF16, tag="g")
                        nc.vector.tensor_tensor(
                            g[:], t1[:],
                            silu_sb[:, m, mb * 512 : mb * 512 + 512], AL.mult,
                        )
                        for i_m in range(4):
                            for j in range(2):
                                nc.tensor.matmul(
                                    ps_o[i_m * 2 + j][:],
                                    g[:, i_m * P : (i_m + 1) * P],
                                    wo_rd[:, j * 512 : j * 512 + 512],
                                    start=(m == 0),
                                    stop=(m == MT - 1),
                                )
                    for i_m in range(4):
                        for j in range(2):
                            o_sb = wk.tile([P, 512], F32, tag="osb")
                            if j == 0:
                                nc.scalar.copy(o_sb[:], ps_o[i_m * 2 + j][:])
                            else:
                                nc.vector.tensor_copy(o_sb[:], ps_o[i_m * 2 + j][:])
                            nc.sync.dma_start(
                                out_d[
                                    (mb * 4 + i_m) * P : (mb * 4 + i_m + 1) * P,
                                    nbb * 1024 + j * 512 : nbb * 1024 + j * 512 + 512,
                                ],
                                o_sb[:],
                            )
            scopeD.__exit__(None, None, None)
    nc.compile()
    return nc


def _prep_inputs(x, cu_seqlens, w_w, wz_w, wh_w, wo_w, conv_w, NC, fp8_z=FP8_Z):
    """Host-side sharding + layout prep. Returns in_maps list."""
    S, D = x.shape[1], x.shape[2]
    H = w_w.shape[0]
    SC = S // NC
    KT, MT = D // P, H // P
    K2 = KT // 2

    xT = np.ascontiguousarray(x[0].T.astype(np.float32))  # (D, S)

    start = np.zeros(S, np.float32)
    for v in np.asarray(cu_seqlens[:-1]):
        v = int(v)
        if 0 <= v < S:
            start[v] = 1.0
    u = 1.0 - start
    u_full = np.ones(S + 2, np.float32)
    u_full[2:] = u  # index j <-> position j-2

    def wprep(wm):  # (H, D) -> (MT, P, KT, P) with [m,p,k,j] = w[m*P+j, k*P+p]
        return np.ascontiguousarray(
            wm.astype(np.float32).reshape(MT, P, KT, P).transpose(0, 3, 2, 1)
        ).astype(BF16)

    wz_f = np.asarray(wz_w, np.float32)
    wh_t, w_t = wprep(wh_w), wprep(w_w)
    if fp8_z:
        wz64 = np.clip(wz_f * WSCALE, -240, 240).astype(FP8)
        wz_t = np.ascontiguousarray(
            wz64.reshape(MT, P, K2, 2, P).transpose(0, 4, 2, 3, 1)
        )
        x16 = np.clip(xT * XSCALE, -240, 240).astype(FP8)
    else:
        wz_t = wprep(wz_w)
    wo_t = np.ascontiguousarray(
        wo_w.T.astype(np.float32).reshape(MT, P, D)
    ).astype(BF16)

    cw_t = conv_w.astype(np.float32)  # (H, CONV)

    in_maps = []
    for c in range(NC):
        s0 = c * SC
        xt_c = np.ascontiguousarray(
            xT[:, s0 : s0 + SC].reshape(KT, P, SC).transpose(1, 0, 2)
        ).astype(BF16)
        # host z_pre history: 3 cols before s0 (zeros at t<0)
        xh = np.zeros((D, 3), np.float32)
        lo = max(0, s0 - 3)
        if s0 > 0:
            xh[:, 3 - (s0 - lo) :] = xT[:, lo:s0]
        zh = wz_f @ xh  # (H, 3)
        czh_c = np.zeros((MT, P, 8), np.float32)
        czh_c[:, :, 0:CONV] = cw_t.reshape(MT, P, CONV)
        czh_c[:, :, 4:7] = zh.reshape(MT, P, 3)
        u_c = np.ascontiguousarray(
            np.broadcast_to(u_full[s0 : s0 + SC + 2], (P, SC + 2))
        )
        sel_c = np.zeros((NC,), np.float32)
        sel_c[c] = 1.0
        sel_c = np.ascontiguousarray(np.broadcast_to(sel_c, (P, NC)))
        imap = {
            "xt": xt_c,
            "wh": wh_t,
            "w": w_t,
            "wo": wo_t,
            "czh": czh_c,
            "u": u_c,
            "sel": sel_c,
        }
        if fp8_z:
            imap["wz8"] = wz_t
            imap["xt8"] = np.ascontiguousarray(
                x16[:, s0 : s0 + SC].reshape(K2, 2, P, SC).transpose(2, 0, 1, 3)
            )
        else:
            imap["wz"] = wz_t
        in_maps.append(imap)
    return in_maps


_NC_CACHE = {}


def run_gru(x, cu_seqlens, w_w, wz_w, wh_w, wo_w, conv_w, NC=8, trace=False):
    S, D = x.shape[1], x.shape[2]
    H = w_w.shape[0]
    SC = S // NC
    key = (D, H, SC, NC, FP8_Z)
    if key not in _NC_CACHE:
        _NC_CACHE[key] = build_gru_kernel(D, H, SC, NC)
    nc = _NC_CACHE[key]
    in_maps = _prep_inputs(x, cu_seqlens, w_w, wz_w, wh_w, wo_w, conv_w, NC)
    res = run_bass_kernel_spmd(nc, in_maps, list(range(NC)), trace=trace)
    out = np.concatenate([res.results[c]["out"] for c in range(NC)], axis=0)
    return out.reshape(1, S, D).astype(np.float32), res


def kernel(**inputs):
    out, _ = run_gru(
        inputs["x"],
        inputs["cu_seqlens"],
        inputs["w_w"],
        inputs["wz_w"],
        inputs["wh_w"],
        inputs["wo_w"],
        inputs["conv_w"],
        NC=8,
    )
    return out


# revision 13
# speedup vs baseline: 1.0810x; 1.0808x over previous
"""Trainium2 Bass kernel for nn_GRU_15461882266204 (minGRU with causal conv gate).

Math (reference):
  w0 = x @ w_w.T ; z0 = x @ wz_w.T ; th = x @ wh_w.T          (S,H)
  z  = sigmoid(causal_conv4(z0, conv_w, segment-masked))
  a  = (1-z) * (1-start) ; b = z * th
  h_t = a_t * h_{t-1} + b_t                                    (scan over S)
  out = (h * silu(w0)) @ wo_w.T                                (S,D)

Strategy: sequence-parallel over 8 NeuronCores (1024 positions each, all 5632
channels per core). Phases:
  A1: per hidden m-tile: z/ht projections (bf16 or fp8-DoubleRow for z),
      causal conv + gates on DVE with host-precomputed boundary masks and
      host-precomputed 3-column z_pre history (kills all narrow matmuls),
      hardware tensor_tensor_scan for h_loc and the cumprod A. h_loc/A go to
      DRAM in bf16 (one merged DMA); chunk-end summaries stay in SBUF.
  B:  360KB AllGather of (A_end, h_end) summaries; every core redundantly
      computes the carry chain. Runs concurrently with A2.
  A2: w0 projection + silu, kept resident in SBUF (bf16).
  D:  fused carry fixup + down-projection: per output block, per m-tile:
      g = (h_loc + A*carry) * silu on DVE feeding bf16 matmuls accumulating
      over all 44 m-tiles in PSUM. Output is sequence-sharded; host concats.
"""
import sys

sys.path.insert(0, "/opt/trn_rl_repo")

import numpy as np

import concourse.bacc as bacc
import concourse.mybir as mybir
import concourse.tile as tile
from concourse.bass_utils import run_bass_kernel_spmd

import ml_dtypes

BF16 = np.dtype(ml_dtypes.bfloat16)
FP8 = np.dtype(ml_dtypes.float8_e4m3)

F32 = mybir.dt.float32
MBF16 = mybir.dt.bfloat16
MFP8 = mybir.dt.float8e4
AL = mybir.AluOpType
ACTF = mybir.ActivationFunctionType
PERF_DR = mybir.MatmulPerfMode.DoubleRow

P = 128
CONV = 4
# fp8 (e4m3, DoubleRow) for the z projection: its error is damped by the
# sigmoid; x scaled by 16, wz by 64 on host, descaled at PSUM drain.
FP8_Z = False
XSCALE = 16.0
WSCALE = 64.0
DESCALE = 1.0 / (XSCALE * WSCALE)


def build_gru_kernel(D, H, SC, NC, fp8_z=FP8_Z):
    KT = D // P          # contraction k-tiles (16)
    K2 = KT // 2         # fp8 DoubleRow k-tiles (8)
    MT = H // P          # hidden m-tiles (44)
    SCH = SC + 3         # z_pre cols incl 3 history cols
    MPT = SC // P        # seq row-tiles (8)

    nc = bacc.Bacc(None, target_bir_lowering=False, debug=False)

    xt_in = nc.declare_dram_parameter("xt", [P, KT, SC], MBF16, isOutput=False)
    wz_in = (
        nc.declare_dram_parameter("wz8", [MT, P, K2, 2, P], MFP8, isOutput=False)
        if fp8_z
        else nc.declare_dram_parameter("wz", [MT, P, KT, P], MBF16, isOutput=False)
    )
    if fp8_z:
        xt8_in = nc.declare_dram_parameter(
            "xt8", [P, K2, 2, SC], MFP8, isOutput=False
        )
    wh_in = nc.declare_dram_parameter("wh", [MT, P, KT, P], MBF16, isOutput=False)
    w_in = nc.declare_dram_parameter("w", [MT, P, KT, P], MBF16, isOutput=False)
    wo_in = nc.declare_dram_parameter("wo", [MT, P, D], MBF16, isOutput=False)
    # czh[m]: cols 0-3 conv_w taps, 4-6 host z_pre history, 7 pad
    czh_in = nc.declare_dram_parameter("czh", [MT, P, 8], F32, isOutput=False)
    u_in = nc.declare_dram_parameter("u", [P, SC + 2], F32, isOutput=False)
    sel_in = nc.declare_dram_parameter("sel", [P, NC], F32, isOutput=False)
    out_d = nc.declare_dram_parameter("out", [SC, D], F32, isOutput=True)

    with tile.TileContext(nc) as tc:
        with (
            tc.tile_pool(name="const", bufs=1) as cpool,
            tc.tile_pool(name="wts", bufs=2) as wpool,
            tc.tile_pool(name="work", bufs=2) as wk,
            tc.tile_pool(name="dload", bufs=3) as dl,
            tc.tile_pool(name="psum", bufs=8, space="PSUM") as pp,
            tc.tile_pool(name="dram", bufs=1, space="DRAM") as dp,
        ):
            # ---- resident tiles ------------------------------------------------
            # xt on the sync queue; u/sel on the scalar (Activation) HWDGE
            # queue so the first m-tile's weight DMAs start right behind xt.
            xt_sb = cpool.tile([P, KT, SC], MBF16, tag="xt")
            nc.sync.dma_start(xt_sb[:, :, 0:512], xt_in[:, :, 0:512])
            nc.sync.dma_start(xt_sb[:, :, 512:SC], xt_in[:, :, 512:SC])
            if fp8_z:
                xt8_sb = cpool.tile([P, K2, 2, SC], MFP8, tag="xt8")
                nc.scalar.dma_start(xt8_sb[:], xt8_in[:])
            u_sb = cpool.tile([P, SC + 2], F32, tag="u")
            nc.scalar.dma_start(u_sb[:], u_in[:])
            sel_sb = cpool.tile([P, NC], F32, tag="sel")
            nc.scalar.dma_start(sel_sb[:], sel_in[:])
            summA = cpool.tile([P, MT], F32, tag="summA")
            summH = cpool.tile([P, MT], F32, tag="summH")
            silu_sb = cpool.tile([P, MT, SC], MBF16, tag="silu")

            # internal DRAM bounce buffers
            hlA_d = dp.tile([MT, P, 2, SC], MBF16)  # [:,0,:]=A  [:,1,:]=h_loc
            summ_d = dp.tile([P, 2 * MT], F32)
            gath_d = dp.tile([NC, P, 2 * MT], F32, addr_space="Shared")

            # ---- phase A1: z/ht matmuls, conv, gating, local scans -------------
            scopeA = nc.named_scope("phaseA1"); scopeA.__enter__()
            for m in range(MT):
                czh = wk.tile([P, 8], F32, tag="czh")
                nc.sync.dma_start(czh[:], czh_in[m])
                if fp8_z:
                    wz_sb = wpool.tile([P, K2, 2, P], MFP8, tag="wz8")
                else:
                    wz_sb = wpool.tile([P, KT, P], MBF16, tag="wz")
                nc.sync.dma_start(wz_sb[:], wz_in[m])
                wh_sb = wpool.tile([P, KT, P], MBF16, tag="wh")
                nc.sync.dma_start(wh_sb[:], wh_in[m])

                # z_pre: cols 0-2 = host history, cols 3.. = matmul
                zp = wk.tile([P, SCH], F32, tag="zpre")
                nc.scalar.copy(zp[:, 0:3], czh[:, 4:7])
                for half in (0, 1):
                    ps = pp.tile([P, 512], F32, tag="ps")
                    if fp8_z:
                        for k2 in range(K2):
                            nc.tensor.matmul(
                                ps[:],
                                wz_sb[:, k2],
                                xt8_sb[:, k2, :, half * 512 : half * 512 + 512],
                                start=(k2 == 0),
                                stop=(k2 == K2 - 1),
                                perf_mode=PERF_DR,
                            )
                        nc.scalar.activation(
                            zp[:, 3 + half * 512 : 3 + half * 512 + 512],
                            ps[:],
                            ACTF.Copy,
                            scale=DESCALE,
                        )
                    else:
                        for k in range(KT):
                            nc.tensor.matmul(
                                ps[:],
                                wz_sb[:, k],
                                xt_sb[:, k, half * 512 : half * 512 + 512],
                                start=(k == 0),
                                stop=(k == KT - 1),
                            )
                        nc.scalar.copy(
                            zp[:, 3 + half * 512 : 3 + half * 512 + 512], ps[:]
                        )

                ps_h = []
                for half in (0, 1):
                    ps = pp.tile([P, 512], F32, tag="ps")
                    for k in range(KT):
                        nc.tensor.matmul(
                            ps[:],
                            wh_sb[:, k],
                            xt_sb[:, k, half * 512 : half * 512 + 512],
                            start=(k == 0),
                            stop=(k == KT - 1),
                        )
                    ps_h.append(ps)

                # conv: yk(t) = u(t)*y{k-1}(t-1), folded in-place into one tile
                y = wk.tile([P, SC + 2], F32, tag="y")
                nc.vector.tensor_tensor(
                    y[:], u_sb[:, : SC + 2], zp[:, : SC + 2], AL.mult
                )
                acc = wk.tile([P, SC], F32, tag="acc")
                nc.vector.tensor_scalar(
                    acc[:], zp[:, 3:SCH], czh[:, 3:4], None, AL.mult
                )
                nc.vector.scalar_tensor_tensor(
                    acc[:], y[:, 2 : SC + 2], czh[:, 2:3], acc[:], AL.mult, AL.add
                )
                nc.vector.tensor_tensor(
                    y[:, : SC + 1], u_sb[:, 1 : SC + 2], y[:, : SC + 1], AL.mult
                )
                nc.vector.scalar_tensor_tensor(
                    acc[:], y[:, 1 : SC + 1], czh[:, 1:2], acc[:], AL.mult, AL.add
                )
                nc.vector.tensor_tensor(
                    y[:, :SC], u_sb[:, 2 : SC + 2], y[:, :SC], AL.mult
                )
                nc.vector.scalar_tensor_tensor(
                    acc[:], y[:, :SC], czh[:, 0:1], acc[:], AL.mult, AL.add
                )

                zt = wk.tile([P, SC], F32, tag="zt")
                nc.scalar.activation(zt[:], acc[:], ACTF.Sigmoid)
                na = wk.tile([P, SC + 2], F32, tag="y", name="na")
                nc.scalar.activation(na[:, :SC], acc[:], ACTF.Sigmoid, scale=-1.0)
                # a = (1-z)*u, in place
                nc.vector.tensor_tensor(
                    na[:, :SC], na[:, :SC], u_sb[:, 2 : SC + 2], AL.mult
                )
                # b = z*th, in place over zt
                for half, ps in zip((0, 1), ps_h):
                    nc.vector.tensor_tensor(
                        zt[:, half * 512 : half * 512 + 512],
                        zt[:, half * 512 : half * 512 + 512],
                        ps[:],
                        AL.mult,
                    )

                hlA = wk.tile([P, 2, SC], MBF16, tag="hlA")
                nc.vector.tensor_tensor_scan(
                    hlA[:, 1, :], na[:, :SC], zt[:], 0.0, AL.mult, AL.add
                )
                # A-scan: u==1 wherever a!=0, so a*u*A == a*A
                nc.vector.tensor_tensor_scan(
                    hlA[:, 0, :], na[:, :SC], u_sb[:, 2 : SC + 2], 1.0,
                    AL.mult, AL.mult,
                )
                nc.scalar.copy(summA[:, m : m + 1], hlA[:, 0, SC - 1 : SC])
                nc.scalar.copy(summH[:, m : m + 1], hlA[:, 1, SC - 1 : SC])
                nc.sync.dma_start(hlA_d[m], hlA[:])
            scopeA.__exit__(None, None, None)

            # ---- phase B (launch): AllGather of scan summaries -----------------
            # Only the summary DMAs + the collective itself go here (gpsimd
            # queue) so nothing downstream head-blocks the sync queue while
            # the collective is in flight; the gather readback + carry chain
            # are emitted after phase A2.
            nc.sync.dma_start(summ_d[:, 0:MT], summA[:])
            nc.sync.dma_start(summ_d[:, MT : 2 * MT], summH[:])
            nc.gpsimd.collective_compute(
                "AllGather",
                AL.bypass,
                replica_groups=[list(range(NC))],
                ins=[summ_d.opt()],
                outs=[gath_d.opt()],
            )

            # ---- phase A2: w0 projection + silu, resident in SBUF --------------
            scopeC = nc.named_scope("phaseA2"); scopeC.__enter__()
            for m in range(MT):
                w_sb = wpool.tile([P, KT, P], MBF16, tag="wh", name="w_sb")
                nc.sync.dma_start(w_sb[:], w_in[m])
                for half in (0, 1):
                    ps = pp.tile([P, 512], F32, tag="ps")
                    for k in range(KT):
                        nc.tensor.matmul(
                            ps[:],
                            w_sb[:, k],
                            xt_sb[:, k, half * 512 : half * 512 + 512],
                            start=(k == 0),
                            stop=(k == KT - 1),
                        )
                    nc.scalar.activation(
                        silu_sb[:, m, half * 512 : half * 512 + 512],
                        ps[:],
                        ACTF.Silu,
                    )
            scopeC.__exit__(None, None, None)

            # ---- phase B (consume): gather readback + redundant carry chain ----
            gsum = []
            for r in range(NC):
                gs = cpool.tile([P, 2 * MT], F32, tag=f"gsum{r}", name=f"gsum{r}")
                nc.sync.dma_start(gs[:], gath_d[r])
                gsum.append(gs)
            state = cpool.tile([P, MT], F32, tag="cstate")
            tmp_c = cpool.tile([P, MT], F32, tag="ctmp")
            mycarry = cpool.tile([P, MT], F32, tag="mycarry")
            nc.any.memset(state[:], 0.0)
            nc.any.memset(mycarry[:], 0.0)
            for r in range(NC):
                if r > 0:
                    nc.vector.scalar_tensor_tensor(
                        mycarry[:], state[:], sel_sb[:, r : r + 1], mycarry[:],
                        AL.mult, AL.add,
                    )
                if r < NC - 1:
                    nc.vector.tensor_tensor(
                        tmp_c[:], state[:], gsum[r][:, 0:MT], AL.mult
                    )
                    nc.vector.tensor_tensor(
                        state[:], tmp_c[:], gsum[r][:, MT : 2 * MT], AL.add
                    )

            # ---- phase D: fused carry fixup + down-projection ------------------
            # Flat (block, m) step list with DMA lookahead: loads for the next
            # block are emitted before the previous block's PSUM drain, so
            # drain waits never head-block the load queues (hlA on sync, wo on
            # scalar).
            scopeD = nc.named_scope("phaseD"); scopeD.__enter__()
            steps = [
                (mb, nbb, m) for mb in (0, 1) for nbb in (0, 1) for m in range(MT)
            ]
            LOOK = 2
            loaded = {}

            def issue_load(step):
                mb, nbb, m = step
                hlA_rd = dl.tile([P, 2, 512], MBF16, tag="hlard")
                nc.sync.dma_start(
                    hlA_rd[:], hlA_d[m][:, :, mb * 512 : mb * 512 + 512]
                )
                wo_rd = dl.tile([P, 1024], MBF16, tag="word")
                nc.scalar.dma_start(
                    wo_rd[:], wo_in[m][:, nbb * 1024 : nbb * 1024 + 1024]
                )
                loaded[step] = (hlA_rd, wo_rd)

            for j0 in range(LOOK):
                issue_load(steps[j0])
            ps_o = None
            for i, step in enumerate(steps):
                if i + LOOK < len(steps):
                    issue_load(steps[i + LOOK])
                mb, nbb, m = step
                hlA_rd, wo_rd = loaded.pop(step)
                if m == 0:
                    ps_o = [
                        pp.tile([P, 512], F32, tag="ps", name=f"pso{mb}{nbb}{k}")
                        for k in range(8)
                    ]
                t1 = wk.tile([P, 512], MBF16, tag="t1")
                nc.vector.scalar_tensor_tensor(
                    t1[:], hlA_rd[:, 0, :], mycarry[:, m : m + 1],
                    hlA_rd[:, 1, :], AL.mult, AL.add,
                )
                g = wk.tile([P, 512], MBF16, tag="g")
                nc.vector.tensor_tensor(
                    g[:], t1[:], silu_sb[:, m, mb * 512 : mb * 512 + 512], AL.mult
                )
                for i_m in range(4):
                    for j in range(2):
                        nc.tensor.matmul(
                            ps_o[i_m * 2 + j][:],
                            g[:, i_m * P : (i_m + 1) * P],
                            wo_rd[:, j * 512 : j * 512 + 512],
                            start=(m == 0),
                            stop=(m == MT - 1),
                        )
                if m == MT - 1:
                    # drain on scalar+vector; out DMAs from the scalar queue
                    for i_m in range(4):
                        for j in range(2):
                            o_sb = wk.tile([P, 512], F32, tag="osb")
                            if j == 0:
                                nc.scalar.copy(o_sb[:], ps_o[i_m * 2 + j][:])
                            else:
                                nc.vector.tensor_copy(o_sb[:], ps_o[i_m * 2 + j][:])
                            nc.scalar.dma_start(
                                out_d[
                                    (mb * 4 + i_m) * P : (mb * 4 + i_m + 1) * P,
                                    nbb * 1024 + j * 512 : nbb * 1024
                                    + j * 512
                                    + 512,
                                ],
                                o_sb[:],
                            )
            scopeD.__exit__(None, None, None)
    nc.compile()
    return nc


def _prep_inputs(x, cu_seqlens, w_w, wz_w, wh_w, wo_w, conv_w, NC, fp8_z=FP8_Z):
    """Host-side sharding + layout prep. Returns in_maps list."""
    S, D = x.shape[1], x.shape[2]
    H = w_w.shape[0]
    SC = S // NC
    KT, MT = D // P, H // P
    K2 = KT // 2

    xT = np.ascontiguousarray(x[0].T.astype(np.float32))  # (D, S)

    start = np.zeros(S, np.float32)
    for v in np.asarray(cu_seqlens[:-1]):
        v = int(v)
        if 0 <= v < S:
            start[v] = 1.0
    u = 1.0 - start
    u_full = np.ones(S + 2, np.float32)
    u_full[2:] = u  # index j <-> position j-2

    def wprep(wm):  # (H, D) -> (MT, P, KT, P) with [m,p,k,j] = w[m*P+j, k*P+p]
        return np.ascontiguousarray(
            wm.astype(np.float32).reshape(MT, P, KT, P).transpose(0, 3, 2, 1)
        ).astype(BF16)

    wz_f = np.asarray(wz_w, np.float32)
    wh_t, w_t = wprep(wh_w), wprep(w_w)
    if fp8_z:
        wz64 = np.clip(wz_f * WSCALE, -240, 240).astype(FP8)
        wz_t = np.ascontiguousarray(
            wz64.reshape(MT, P, K2, 2, P).transpose(0, 4, 2, 3, 1)
        )
        x16 = np.clip(xT * XSCALE, -240, 240).astype(FP8)
    else:
        wz_t = wprep(wz_w)
    wo_t = np.ascontiguousarray(
        wo_w.T.astype(np.float32).reshape(MT, P, D)
    ).astype(BF16)

    cw_t = conv_w.astype(np.float32)  # (H, CONV)

    in_maps = []
    for c in range(NC):
        s0 = c * SC
        xt_c = np.ascontiguousarray(
            xT[:, s0 : s0 + SC].reshape(KT, P, SC).transpose(1, 0, 2)
        ).astype(BF16)
        # host z_pre history: 3 cols before s0 (zeros at t<0)
        xh = np.zeros((D, 3), np.float32)
        lo = max(0, s0 - 3)
        if s0 > 0:
            xh[:, 3 - (s0 - lo) :] = xT[:, lo:s0]
        zh = wz_f @ xh  # (H, 3)
        czh_c = np.zeros((MT, P, 8), np.float32)
        czh_c[:, :, 0:CONV] = cw_t.reshape(MT, P, CONV)
        czh_c[:, :, 4:7] = zh.reshape(MT, P, 3)
        u_c = np.ascontiguousarray(
            np.broadcast_to(u_full[s0 : s0 + SC + 2], (P, SC + 2))
        )
        sel_c = np.zeros((NC,), np.float32)
        sel_c[c] = 1.0
        sel_c = np.ascontiguousarray(np.broadcast_to(sel_c, (P, NC)))
        imap = {
            "xt": xt_c,
            "wh": wh_t,
            "w": w_t,
            "wo": wo_t,
            "czh": czh_c,
            "u": u_c,
            "sel": sel_c,
        }
        if fp8_z:
            imap["wz8"] = wz_t
            imap["xt8"] = np.ascontiguousarray(
                x16[:, s0 : s0 + SC].reshape(K2, 2, P, SC).transpose(2, 0, 1, 3)
            )
        else:
            imap["wz"] = wz_t
        in_maps.append(imap)
    return in_maps


_NC_CACHE = {}


def run_gru(x, cu_seqlens, w_w, wz_w, wh_w, wo_w, conv_w, NC=8, trace=False):
    S, D = x.shape[1], x.shape[2]
    H = w_w.shape[0]
    SC = S // NC
    key = (D, H, SC, NC, FP8_Z)
    if key not in _NC_CACHE:
        _NC_CACHE[key] = build_gru_kernel(D, H, SC, NC)
    nc = _NC_CACHE[key]
    in_maps = _prep_inputs(x, cu_seqlens, w_w, wz_w, wh_w, wo_w, conv_w, NC)
    res = run_bass_kernel_spmd(nc, in_maps, list(range(NC)), trace=trace)
    out = np.concatenate([res.results[c]["out"] for c in range(NC)], axis=0)
    return out.reshape(1, S, D).astype(np.float32), res


def kernel(**inputs):
    out, _ = run_gru(
        inputs["x"],
        inputs["cu_seqlens"],
        inputs["w_w"],
        inputs["wz_w"],
        inputs["wh_w"],
        inputs["wo_w"],
        inputs["conv_w"],
        NC=8,
    )
    return out


# revision 24
# speedup vs baseline: 1.1389x; 1.0536x over previous
"""Trainium2 Bass kernel for nn_GRU_15461882266204 (minGRU with causal conv gate).

Math (reference):
  w0 = x @ w_w.T ; z0 = x @ wz_w.T ; th = x @ wh_w.T          (S,H)
  z  = sigmoid(causal_conv4(z0, conv_w, segment-masked))
  a  = (1-z) * (1-start) ; b = z * th
  h_t = a_t * h_{t-1} + b_t                                    (scan over S)
  out = (h * silu(w0)) @ wo_w.T                                (S,D)

Strategy: sequence-parallel over 8 NeuronCores (1024 positions each, all 5632
channels per core). Phases:
  A1: per hidden m-tile: z/ht projections (bf16 or fp8-DoubleRow for z),
      causal conv + gates on DVE with host-precomputed boundary masks and
      host-precomputed 3-column z_pre history (kills all narrow matmuls),
      hardware tensor_tensor_scan for h_loc and the cumprod A. h_loc/A go to
      DRAM in bf16 (one merged DMA); chunk-end summaries stay in SBUF.
  B:  360KB AllGather of (A_end, h_end) summaries; every core redundantly
      computes the carry chain. Runs concurrently with A2.
  A2: w0 projection + silu, kept resident in SBUF (bf16).
  D:  fused carry fixup + down-projection: per output block, per m-tile:
      g = (h_loc + A*carry) * silu on DVE feeding bf16 matmuls accumulating
      over all 44 m-tiles in PSUM. Output is sequence-sharded; host concats.
"""
import sys

sys.path.insert(0, "/opt/trn_rl_repo")

import numpy as np

import concourse.bacc as bacc
import concourse.mybir as mybir
import concourse.tile as tile
from concourse.bass_utils import run_bass_kernel_spmd

import ml_dtypes

BF16 = np.dtype(ml_dtypes.bfloat16)
FP8 = np.dtype(ml_dtypes.float8_e4m3)

F32 = mybir.dt.float32
MBF16 = mybir.dt.bfloat16
MFP8 = mybir.dt.float8e4
AL = mybir.AluOpType
ACTF = mybir.ActivationFunctionType
PERF_DR = mybir.MatmulPerfMode.DoubleRow

P = 128
CONV = 4
# fp8 (e4m3, DoubleRow) for the z projection: its error is damped by the
# sigmoid; x scaled by 16, wz by 64 on host, descaled at PSUM drain.
FP8_Z = False
XSCALE = 16.0
WSCALE = 64.0
DESCALE = 1.0 / (XSCALE * WSCALE)


def build_gru_kernel(D, H, SC, NC, fp8_z=FP8_Z):
    KT = D // P          # contraction k-tiles (16)
    K2 = KT // 2         # fp8 DoubleRow k-tiles (8)
    MT = H // P          # hidden m-tiles (44)
    SCH = SC + 3         # z_pre cols incl 3 history cols
    MPT = SC // P        # seq row-tiles (8)

    nc = bacc.Bacc(None, target_bir_lowering=False, debug=False)

    xt_in = nc.declare_dram_parameter("xt", [P, KT, SC], MBF16, isOutput=False)
    wz_in = (
        nc.declare_dram_parameter("wz8", [MT, P, K2, 2, P], MFP8, isOutput=False)
        if fp8_z
        else nc.declare_dram_parameter("wz", [MT, P, KT, P], MBF16, isOutput=False)
    )
    if fp8_z:
        xt8_in = nc.declare_dram_parameter(
            "xt8", [P, K2, 2, SC], MFP8, isOutput=False
        )
    wh_in = nc.declare_dram_parameter("wh", [MT, P, KT, P], MBF16, isOutput=False)
    w_in = nc.declare_dram_parameter("w", [MT, P, KT, P], MBF16, isOutput=False)
    wo_in = nc.declare_dram_parameter("wo", [MT, P, D], MBF16, isOutput=False)
    # czh[:,m]: cols 0-3 conv_w taps, 4-6 host z_pre history, 7 pad
    czh_in = nc.declare_dram_parameter("czh", [P, MT, 8], F32, isOutput=False)
    u_in = nc.declare_dram_parameter("u", [P, SC + 2], MBF16, isOutput=False)
    sel_in = nc.declare_dram_parameter("sel", [P, NC], F32, isOutput=False)
    out_d = nc.declare_dram_parameter("out", [SC, D], F32, isOutput=True)

    with tile.TileContext(nc) as tc:
        with (
            tc.tile_pool(name="const", bufs=1) as cpool,
            tc.tile_pool(name="wts", bufs=2) as wpool,
            tc.tile_pool(name="work", bufs=2) as wk,
            tc.tile_pool(name="dload", bufs=4) as dl,
            tc.tile_pool(name="psum", bufs=8, space="PSUM") as pp,
            tc.tile_pool(name="dram", bufs=1, space="DRAM") as dp,
        ):
            # ---- resident tiles ------------------------------------------------
            # xt on the sync queue; u/sel on the scalar (Activation) HWDGE
            # queue so the first m-tile's weight DMAs start right behind xt.
            xt_sb = cpool.tile([P, KT, SC], MBF16, tag="xt")
            nc.sync.dma_start(xt_sb[:, :, 0:512], xt_in[:, :, 0:512])
            nc.scalar.dma_start(xt_sb[:, :, 512:SC], xt_in[:, :, 512:SC])
            if fp8_z:
                xt8_sb = cpool.tile([P, K2, 2, SC], MFP8, tag="xt8")
                nc.scalar.dma_start(xt8_sb[:], xt8_in[:])
            u_sb = cpool.tile([P, SC + 2], MBF16, tag="u")
            nc.scalar.dma_start(u_sb[:], u_in[:])
            sel_sb = cpool.tile([P, NC], F32, tag="sel")
            nc.scalar.dma_start(sel_sb[:], sel_in[:])
            # all 44 m-tiles' conv taps + z history in ONE dma (tiny rows are
            # descriptor-bound: 44 separate [P,8] DMAs cost ~8us each)
            czh_sb = cpool.tile([P, MT, 8], F32, tag="czh")
            nc.scalar.dma_start(czh_sb[:], czh_in[:])
            summA = cpool.tile([P, MT], F32, tag="summA")
            summH = cpool.tile([P, MT], F32, tag="summH")
            silu_sb = cpool.tile([P, MT, SC], MBF16, tag="silu")

            # internal DRAM bounce buffers
            hlA_d = dp.tile([MT, P, 2, SC], MBF16)  # [:,0,:]=A  [:,1,:]=h_loc
            summ_d = dp.tile([P, 2 * MT], F32)
            gath_d = dp.tile([NC, P, 2 * MT], F32, addr_space="Shared")

            # ---- phase A1: z/ht matmuls, conv, gating, local scans -------------
            scopeA = nc.named_scope("phaseA1"); scopeA.__enter__()
            for m in range(MT):
                czh = czh_sb[:, m]
                if fp8_z:
                    wz_sb = wpool.tile([P, K2, 2, P], MFP8, tag="wz8")
                else:
                    wz_sb = wpool.tile([P, KT, P], MBF16, tag="wz")
                nc.sync.dma_start(wz_sb[:], wz_in[m])
                wh_sb = wpool.tile([P, KT, P], MBF16, tag="wh")
                nc.sync.dma_start(wh_sb[:], wh_in[m])

                # z_pre: cols 0-2 = host history, cols 3.. = matmul
                zp = wk.tile([P, SCH], F32, tag="zpre")
                nc.scalar.copy(zp[:, 0:3], czh[:, 4:7])
                for half in (0, 1):
                    ps = pp.tile([P, 512], F32, tag="ps")
                    if fp8_z:
                        for k2 in range(K2):
                            nc.tensor.matmul(
                                ps[:],
                                wz_sb[:, k2],
                                xt8_sb[:, k2, :, half * 512 : half * 512 + 512],
                                start=(k2 == 0),
                                stop=(k2 == K2 - 1),
                                perf_mode=PERF_DR,
                            )
                        nc.scalar.activation(
                            zp[:, 3 + half * 512 : 3 + half * 512 + 512],
                            ps[:],
                            ACTF.Copy,
                            scale=DESCALE,
                        )
                    else:
                        for k in range(KT):
                            nc.tensor.matmul(
                                ps[:],
                                wz_sb[:, k],
                                xt_sb[:, k, half * 512 : half * 512 + 512],
                                start=(k == 0),
                                stop=(k == KT - 1),
                            )
                        nc.scalar.copy(
                            zp[:, 3 + half * 512 : 3 + half * 512 + 512], ps[:]
                        )

                ps_h = []
                for half in (0, 1):
                    ps = pp.tile([P, 512], F32, tag="ps")
                    for k in range(KT):
                        nc.tensor.matmul(
                            ps[:],
                            wh_sb[:, k],
                            xt_sb[:, k, half * 512 : half * 512 + 512],
                            start=(k == 0),
                            stop=(k == KT - 1),
                        )
                    ps_h.append(ps)

                # conv: yk(t) = u(t)*y{k-1}(t-1), folded in-place into one tile
                y = wk.tile([P, SC + 2], F32, tag="y")
                nc.vector.tensor_tensor(
                    y[:], u_sb[:, : SC + 2], zp[:, : SC + 2], AL.mult
                )
                acc = wk.tile([P, SC], F32, tag="acc")
                nc.vector.tensor_scalar(
                    acc[:], zp[:, 3:SCH], czh[:, 3:4], None, AL.mult
                )
                nc.vector.scalar_tensor_tensor(
                    acc[:], y[:, 2 : SC + 2], czh[:, 2:3], acc[:], AL.mult, AL.add
                )
                nc.vector.tensor_tensor(
                    y[:, : SC + 1], u_sb[:, 1 : SC + 2], y[:, : SC + 1], AL.mult
                )
                nc.vector.scalar_tensor_tensor(
                    acc[:], y[:, 1 : SC + 1], czh[:, 1:2], acc[:], AL.mult, AL.add
                )
                nc.vector.tensor_tensor(
                    y[:, :SC], u_sb[:, 2 : SC + 2], y[:, :SC], AL.mult
                )
                nc.vector.scalar_tensor_tensor(
                    acc[:], y[:, :SC], czh[:, 0:1], acc[:], AL.mult, AL.add
                )

                zt = wk.tile([P, SC], F32, tag="zt")
                nc.scalar.activation(zt[:], acc[:], ACTF.Sigmoid)
                na = wk.tile([P, SC + 2], F32, tag="y", name="na")
                nc.scalar.activation(na[:, :SC], acc[:], ACTF.Sigmoid, scale=-1.0)
                # a = (1-z)*u, in place
                nc.vector.tensor_tensor(
                    na[:, :SC], na[:, :SC], u_sb[:, 2 : SC + 2], AL.mult
                )
                # b = z*th, in place over zt
                for half, ps in zip((0, 1), ps_h):
                    nc.vector.tensor_tensor(
                        zt[:, half * 512 : half * 512 + 512],
                        zt[:, half * 512 : half * 512 + 512],
                        ps[:],
                        AL.mult,
                    )

                hlA = wk.tile([P, 2, SC], MBF16, tag="hlA")
                nc.vector.tensor_tensor_scan(
                    hlA[:, 1, :], na[:, :SC], zt[:], 0.0, AL.mult, AL.add
                )
                # A-scan: u==1 wherever a!=0, so a*u*A == a*A
                nc.vector.tensor_tensor_scan(
                    hlA[:, 0, :], na[:, :SC], u_sb[:, 2 : SC + 2], 1.0,
                    AL.mult, AL.mult,
                )
                nc.scalar.copy(summA[:, m : m + 1], hlA[:, 0, SC - 1 : SC])
                nc.scalar.copy(summH[:, m : m + 1], hlA[:, 1, SC - 1 : SC])
                nc.sync.dma_start(hlA_d[m], hlA[:])
            scopeA.__exit__(None, None, None)

            # ---- phase B (launch): AllGather of scan summaries -----------------
            # Only the summary DMAs + the collective itself go here (gpsimd
            # queue) so nothing downstream head-blocks the sync queue while
            # the collective is in flight; the gather readback + carry chain
            # are emitted after phase A2.
            nc.scalar.dma_start(summ_d[:, 0:MT], summA[:])
            nc.scalar.dma_start(summ_d[:, MT : 2 * MT], summH[:])
            nc.gpsimd.collective_compute(
                "AllGather",
                AL.bypass,
                replica_groups=[list(range(NC))],
                ins=[summ_d.opt()],
                outs=[gath_d.opt()],
            )

            # ---- phase A2: w0 projection + silu, resident in SBUF --------------
            # The carry-chain consumption (gather readback + 16 tiny DVE ops)
            # is emitted mid-loop, once the collective is certainly complete,
            # so phase D starts unblocked the moment A2's matmuls finish.
            state = cpool.tile([P, MT], F32, tag="cstate")
            tmp_c = cpool.tile([P, MT], F32, tag="ctmp")
            mycarry = cpool.tile([P, MT], F32, tag="mycarry")
            scopeC = nc.named_scope("phaseA2"); scopeC.__enter__()
            for m in range(MT):
                w_sb = wpool.tile([P, KT, P], MBF16, tag="wh", name="w_sb")
                nc.sync.dma_start(w_sb[:], w_in[m])
                for half in (0, 1):
                    ps = pp.tile([P, 512], F32, tag="ps")
                    for k in range(KT):
                        nc.tensor.matmul(
                            ps[:],
                            w_sb[:, k],
                            xt_sb[:, k, half * 512 : half * 512 + 512],
                            start=(k == 0),
                            stop=(k == KT - 1),
                        )
                    nc.scalar.activation(
                        silu_sb[:, m, half * 512 : half * 512 + 512],
                        ps[:],
                        ACTF.Silu,
                    )
                if m == 11:
                    # ---- phase B (consume): redundant carry chain --------------
                    nc.any.memset(state[:], 0.0)
                    nc.any.memset(mycarry[:], 0.0)
                    for r in range(NC):
                        gs = wk.tile([P, SC], F32, tag="acc", name=f"gs{r}")
                        nc.sync.dma_start(gs[:, 0 : 2 * MT], gath_d[r])
                        if r > 0:
                            nc.vector.scalar_tensor_tensor(
                                mycarry[:], state[:], sel_sb[:, r : r + 1],
                                mycarry[:], AL.mult, AL.add,
                            )
                        if r < NC - 1:
                            nc.vector.tensor_tensor(
                                tmp_c[:], state[:], gs[:, 0:MT], AL.mult
                            )
                            nc.vector.tensor_tensor(
                                state[:], tmp_c[:], gs[:, MT : 2 * MT], AL.add
                            )
            scopeC.__exit__(None, None, None)

            # ---- phase D: fused carry fixup + down-projection ------------------
            # Flat (block, m) step list with DMA lookahead: loads for the next
            # block are emitted before the previous block's PSUM drain, so
            # drain waits never head-block the load queues (hlA on sync, wo on
            # scalar).
            scopeD = nc.named_scope("phaseD"); scopeD.__enter__()
            steps = [
                (mb, nbb, m) for mb in (0, 1) for nbb in (0, 1) for m in range(MT)
            ]
            LOOK = 3
            loaded = {}

            def issue_load(step):
                mb, nbb, m = step
                hlA_rd = dl.tile([P, 2, 512], MBF16, tag="hlard")
                nc.sync.dma_start(
                    hlA_rd[:], hlA_d[m][:, :, mb * 512 : mb * 512 + 512]
                )
                wo_rd = dl.tile([P, 1024], MBF16, tag="word")
                nc.scalar.dma_start(
                    wo_rd[:], wo_in[m][:, nbb * 1024 : nbb * 1024 + 1024]
                )
                loaded[step] = (hlA_rd, wo_rd)

            for j0 in range(LOOK):
                issue_load(steps[j0])
            ps_o = None
            for i, step in enumerate(steps):
                if i + LOOK < len(steps):
                    issue_load(steps[i + LOOK])
                mb, nbb, m = step
                hlA_rd, wo_rd = loaded.pop(step)
                if m == 0:
                    ps_o = [
                        pp.tile([P, 512], F32, tag="ps", name=f"pso{mb}{nbb}{k}")
                        for k in range(8)
                    ]
                t1 = wk.tile([P, 512], MBF16, tag="t1")
                nc.vector.scalar_tensor_tensor(
                    t1[:], hlA_rd[:, 0, :], mycarry[:, m : m + 1],
                    hlA_rd[:, 1, :], AL.mult, AL.add,
                )
                g = wk.tile([P, 512], MBF16, tag="g")
                nc.vector.tensor_tensor(
                    g[:], t1[:], silu_sb[:, m, mb * 512 : mb * 512 + 512], AL.mult
                )
                for i_m in range(4):
                    for j in range(2):
                        nc.tensor.matmul(
                            ps_o[i_m * 2 + j][:],
                            g[:, i_m * P : (i_m + 1) * P],
                            wo_rd[:, j * 512 : j * 512 + 512],
                            start=(m == 0),
                            stop=(m == MT - 1),
                        )
                if m == MT - 1:
                    # drain on scalar+vector; out DMAs from the scalar queue
                    for i_m in range(4):
                        for j in range(2):
                            o_sb = wk.tile([P, 512], F32, tag="osb")
                            if j == 0:
                                nc.scalar.copy(o_sb[:], ps_o[i_m * 2 + j][:])
                            else:
                                nc.vector.tensor_copy(o_sb[:], ps_o[i_m * 2 + j][:])
                            nc.scalar.dma_start(
                                out_d[
                                    (mb * 4 + i_m) * P : (mb * 4 + i_m + 1) * P,
                                    nbb * 1024 + j * 512 : nbb * 1024
                                    + j * 512
                                    + 512,
                                ],
                                o_sb[:],
                            )
            scopeD.__exit__(None, None, None)
    nc.compile()
    return nc


def _prep_inputs(x, cu_seqlens, w_w, wz_w, wh_w, wo_w, conv_w, NC, fp8_z=FP8_Z):
    """Host-side sharding + layout prep. Returns in_maps list."""
    S, D = x.shape[1], x.shape[2]
    H = w_w.shape[0]
    SC = S // NC
    KT, MT = D // P, H // P
    K2 = KT // 2

    xT = np.ascontiguousarray(x[0].T.astype(np.float32))  # (D, S)

    start = np.zeros(S, np.float32)
    for v in np.asarray(cu_seqlens[:-1]):
        v = int(v)
        if 0 <= v < S:
            start[v] = 1.0
    u = 1.0 - start
    u_full = np.ones(S + 2, np.float32)
    u_full[2:] = u  # index j <-> position j-2

    def wprep(wm):  # (H, D) -> (MT, P, KT, P) with [m,p,k,j] = w[m*P+j, k*P+p]
        return np.ascontiguousarray(
            wm.astype(np.float32).reshape(MT, P, KT, P).transpose(0, 3, 2, 1)
        ).astype(BF16)

    wz_f = np.asarray(wz_w, np.float32)
    wh_t, w_t = wprep(wh_w), wprep(w_w)
    if fp8_z:
        wz64 = np.clip(wz_f * WSCALE, -240, 240).astype(FP8)
        wz_t = np.ascontiguousarray(
            wz64.reshape(MT, P, K2, 2, P).transpose(0, 4, 2, 3, 1)
        )
        x16 = np.clip(xT * XSCALE, -240, 240).astype(FP8)
    else:
        wz_t = wprep(wz_w)
    wo_t = np.ascontiguousarray(
        wo_w.T.astype(np.float32).reshape(MT, P, D)
    ).astype(BF16)

    cw_t = conv_w.astype(np.float32)  # (H, CONV)

    in_maps = []
    for c in range(NC):
        s0 = c * SC
        xt_c = np.ascontiguousarray(
            xT[:, s0 : s0 + SC].reshape(KT, P, SC).transpose(1, 0, 2)
        ).astype(BF16)
        # host z_pre history: 3 cols before s0 (zeros at t<0)
        xh = np.zeros((D, 3), np.float32)
        lo = max(0, s0 - 3)
        if s0 > 0:
            xh[:, 3 - (s0 - lo) :] = xT[:, lo:s0]
        zh = wz_f @ xh  # (H, 3)
        czh_c = np.zeros((MT, P, 8), np.float32)
        czh_c[:, :, 0:CONV] = cw_t.reshape(MT, P, CONV)
        czh_c[:, :, 4:7] = zh.reshape(MT, P, 3)
        czh_c = np.ascontiguousarray(czh_c.transpose(1, 0, 2))  # (P, MT, 8)
        u_c = np.ascontiguousarray(
            np.broadcast_to(u_full[s0 : s0 + SC + 2], (P, SC + 2))
        ).astype(BF16)
        sel_c = np.zeros((NC,), np.float32)
        sel_c[c] = 1.0
        sel_c = np.ascontiguousarray(np.broadcast_to(sel_c, (P, NC)))
        imap = {
            "xt": xt_c,
            "wh": wh_t,
            "w": w_t,
            "wo": wo_t,
            "czh": czh_c,
            "u": u_c,
            "sel": sel_c,
        }
        if fp8_z:
            imap["wz8"] = wz_t
            imap["xt8"] = np.ascontiguousarray(
                x16[:, s0 : s0 + SC].reshape(K2, 2, P, SC).transpose(2, 0, 1, 3)
            )
        else:
            imap["wz"] = wz_t
        in_maps.append(imap)
    return in_maps


_NC_CACHE = {}


def run_gru(x, cu_seqlens, w_w, wz_w, wh_w, wo_w, conv_w, NC=8, trace=False):
    S, D = x.shape[1], x.shape[2]
    H = w_w.shape[0]
    SC = S // NC
    key = (D, H, SC, NC, FP8_Z)
    if key not in _NC_CACHE:
        _NC_CACHE[key] = build_gru_kernel(D, H, SC, NC)
    nc = _NC_CACHE[key]
    in_maps = _prep_inputs(x, cu_seqlens, w_w, wz_w, wh_w, wo_w, conv_w, NC)
    res = run_bass_kernel_spmd(nc, in_maps, list(range(NC)), trace=trace)
    out = np.concatenate([res.results[c]["out"] for c in range(NC)], axis=0)
    return out.reshape(1, S, D).astype(np.float32), res


def kernel(**inputs):
    out, _ = run_gru(
        inputs["x"],
        inputs["cu_seqlens"],
        inputs["w_w"],
        inputs["wz_w"],
        inputs["wh_w"],
        inputs["wo_w"],
        inputs["conv_w"],
        NC=8,
    )
    return out


# revision 39
# speedup vs baseline: 1.1485x; 1.0085x over previous
"""Trainium2 Bass kernel for nn_GRU_15461882266204 (minGRU with causal conv gate).

Math (reference):
  w0 = x @ w_w.T ; z0 = x @ wz_w.T ; th = x @ wh_w.T          (S,H)
  z  = sigmoid(causal_conv4(z0, conv_w, segment-masked))
  a  = (1-z) * (1-start) ; b = z * th
  h_t = a_t * h_{t-1} + b_t                                    (scan over S)
  out = (h * silu(w0)) @ wo_w.T                                (S,D)

Strategy: sequence-parallel over 8 NeuronCores (1024 positions each, all 5632
channels per core). Phases:
  A1: per hidden m-tile: z/ht projections (bf16 or fp8-DoubleRow for z),
      causal conv + gates on DVE with host-precomputed boundary masks and
      host-precomputed 3-column z_pre history (kills all narrow matmuls),
      hardware tensor_tensor_scan for h_loc and the cumprod A. h_loc/A go to
      DRAM in bf16 (one merged DMA); chunk-end summaries stay in SBUF.
  B:  360KB AllGather of (A_end, h_end) summaries; every core redundantly
      computes the carry chain. Runs concurrently with A2.
  A2: w0 projection + silu, kept resident in SBUF (bf16).
  D:  fused carry fixup + down-projection: per output block, per m-tile:
      g = (h_loc + A*carry) * silu on DVE feeding bf16 matmuls accumulating
      over all 44 m-tiles in PSUM. Output is sequence-sharded; host concats.
"""
import sys

sys.path.insert(0, "/opt/trn_rl_repo")

import numpy as np

import concourse.bacc as bacc
import concourse.mybir as mybir
import concourse.tile as tile
from concourse.bass_utils import run_bass_kernel_spmd

import ml_dtypes

BF16 = np.dtype(ml_dtypes.bfloat16)
FP8 = np.dtype(ml_dtypes.float8_e4m3)

F32 = mybir.dt.float32
MBF16 = mybir.dt.bfloat16
MFP8 = mybir.dt.float8e4
AL = mybir.AluOpType
ACTF = mybir.ActivationFunctionType
PERF_DR = mybir.MatmulPerfMode.DoubleRow

P = 128
CONV = 4
# fp8 (e4m3, DoubleRow) for the z projection: its error is damped by the
# sigmoid; x scaled by 16, wz by 64 on host, descaled at PSUM drain.
FP8_Z = True
XSCALE = 16.0
WSCALE = 64.0
DESCALE = 1.0 / (XSCALE * WSCALE)


def build_gru_kernel(D, H, SC, NC, fp8_z=FP8_Z):
    KT = D // P          # contraction k-tiles (16)
    K2 = KT // 2         # fp8 DoubleRow k-tiles (8)
    MT = H // P          # hidden m-tiles (44)
    SCH = SC + 3         # z_pre cols incl 3 history cols
    MPT = SC // P        # seq row-tiles (8)

    nc = bacc.Bacc(None, target_bir_lowering=False, debug=False)

    xt_in = nc.declare_dram_parameter("xt", [P, KT, SC], MBF16, isOutput=False)
    wz_in = (
        nc.declare_dram_parameter("wz8", [MT, P, K2, 2, P], MFP8, isOutput=False)
        if fp8_z
        else nc.declare_dram_parameter("wz", [MT, P, KT, P], MBF16, isOutput=False)
    )
    if fp8_z:
        xt8_in = nc.declare_dram_parameter(
            "xt8", [P, K2, 2, SC], MFP8, isOutput=False
        )
    wh_in = nc.declare_dram_parameter("wh", [MT, P, KT, P], MBF16, isOutput=False)
    w_in = nc.declare_dram_parameter("w", [MT, P, KT, P], MBF16, isOutput=False)
    wo_in = nc.declare_dram_parameter("wo", [MT, P, D], MBF16, isOutput=False)
    # czh[:,m]: cols 0-3 conv_w taps, 4-6 host z_pre history, 7 pad
    czh_in = nc.declare_dram_parameter("czh", [P, MT, 8], F32, isOutput=False)
    # padded to 1056 cols: 64B-aligned rows (odd-size DMAs are slow)
    u_in = nc.declare_dram_parameter("u", [P, SC + 32], MBF16, isOutput=False)
    sel_in = nc.declare_dram_parameter("sel", [P, NC], F32, isOutput=False)
    out_d = nc.declare_dram_parameter("out", [SC, D], F32, isOutput=True)

    with tile.TileContext(nc) as tc:
        with (
            tc.tile_pool(name="const", bufs=1) as cpool,
            tc.tile_pool(name="wts", bufs=2) as wpool,
            tc.tile_pool(name="work", bufs=2) as wk,
            tc.tile_pool(name="dload", bufs=3 if fp8_z else 4) as dl,
            tc.tile_pool(name="psum", bufs=8, space="PSUM") as pp,
            tc.tile_pool(name="dram", bufs=1, space="DRAM") as dp,
        ):
            # ---- resident tiles ------------------------------------------------
            # xt on the sync queue; u/sel on the scalar (Activation) HWDGE
            # queue so the first m-tile's weight DMAs start right behind xt.
            xt_sb = cpool.tile([P, KT, SC], MBF16, tag="xt")
            nc.sync.dma_start(xt_sb[:, :, 0:512], xt_in[:, :, 0:512])
            nc.scalar.dma_start(xt_sb[:, :, 512:SC], xt_in[:, :, 512:SC])
            if fp8_z:
                xt8_sb = cpool.tile([P, K2, 2, SC], MFP8, tag="xt8")
                nc.scalar.dma_start(xt8_sb[:], xt8_in[:])
            u_sb = cpool.tile([P, SC + 32], MBF16, tag="u")
            nc.scalar.dma_start(u_sb[:], u_in[:])
            sel_sb = cpool.tile([P, NC], F32, tag="sel")
            nc.scalar.dma_start(sel_sb[:], sel_in[:])
            # all 44 m-tiles' conv taps + z history in ONE dma (tiny rows are
            # descriptor-bound: 44 separate [P,8] DMAs cost ~8us each)
            czh_sb = cpool.tile([P, MT, 8], F32, tag="czh")
            nc.scalar.dma_start(czh_sb[:], czh_in[:])
            summA = cpool.tile([P, 64], F32, tag="summA")
            summH = cpool.tile([P, 64], F32, tag="summH")
            silu_sb = cpool.tile([P, MT, SC], MBF16, tag="silu")

            # internal DRAM bounce buffers
            hlA_d = dp.tile([MT, P, 2, SC], MBF16)  # [:,0,:]=A  [:,1,:]=h_loc
            summ_d = dp.tile([P, 128], F32)
            gath_d = dp.tile([NC, P, 128], F32, addr_space="Shared")

            # ---- phase A1: z/ht matmuls, conv, gating, local scans -------------
            scopeA = nc.named_scope("phaseA1"); scopeA.__enter__()
            for m in range(MT):
                czh = czh_sb[:, m]
                if fp8_z:
                    wz_sb = wpool.tile([P, K2, 2, P], MFP8, tag="wz8")
                else:
                    wz_sb = wpool.tile([P, KT, P], MBF16, tag="wz")
                nc.sync.dma_start(wz_sb[:], wz_in[m])
                wh_sb = wpool.tile([P, KT, P], MBF16, tag="wh")
                nc.sync.dma_start(wh_sb[:], wh_in[m])

                # z_pre: cols 0-2 = host history, cols 3.. = matmul
                zp = wk.tile([P, SCH], MBF16 if fp8_z else F32, tag="zpre")
                nc.scalar.copy(zp[:, 0:3], czh[:, 4:7])
                for half in (0, 1):
                    ps = pp.tile([P, 512], F32, tag="ps")
                    if fp8_z:
                        for k2 in range(K2):
                            nc.tensor.matmul(
                                ps[:],
                                wz_sb[:, k2],
                                xt8_sb[:, k2, :, half * 512 : half * 512 + 512],
                                start=(k2 == 0),
                                stop=(k2 == K2 - 1),
                                perf_mode=PERF_DR,
                            )
                        nc.scalar.activation(
                            zp[:, 3 + half * 512 : 3 + half * 512 + 512],
                            ps[:],
                            ACTF.Copy,
                            scale=DESCALE,
                        )
                    else:
                        for k in range(KT):
                            nc.tensor.matmul(
                                ps[:],
                                wz_sb[:, k],
                                xt_sb[:, k, half * 512 : half * 512 + 512],
                                start=(k == 0),
                                stop=(k == KT - 1),
                            )
                        nc.scalar.copy(
                            zp[:, 3 + half * 512 : 3 + half * 512 + 512], ps[:]
                        )

                ps_h = []
                for half in (0, 1):
                    ps = pp.tile([P, 512], F32, tag="ps")
                    for k in range(KT):
                        nc.tensor.matmul(
                            ps[:],
                            wh_sb[:, k],
                            xt_sb[:, k, half * 512 : half * 512 + 512],
                            start=(k == 0),
                            stop=(k == KT - 1),
                        )
                    ps_h.append(ps)

                # conv: yk(t) = u(t)*y{k-1}(t-1), folded in-place into one tile
                y = wk.tile([P, SC + 2], MBF16 if fp8_z else F32, tag="y")
                nc.vector.tensor_tensor(
                    y[:], u_sb[:, : SC + 2], zp[:, : SC + 2], AL.mult
                )
                acc = wk.tile([P, SC], F32, tag="acc")
                nc.vector.tensor_scalar(
                    acc[:], zp[:, 3:SCH], czh[:, 3:4], None, AL.mult
                )
                nc.vector.scalar_tensor_tensor(
                    acc[:], y[:, 2 : SC + 2], czh[:, 2:3], acc[:], AL.mult, AL.add
                )
                nc.vector.tensor_tensor(
                    y[:, : SC + 1], u_sb[:, 1 : SC + 2], y[:, : SC + 1], AL.mult
                )
                nc.vector.scalar_tensor_tensor(
                    acc[:], y[:, 1 : SC + 1], czh[:, 1:2], acc[:], AL.mult, AL.add
                )
                nc.vector.tensor_tensor(
                    y[:, :SC], u_sb[:, 2 : SC + 2], y[:, :SC], AL.mult
                )
                nc.vector.scalar_tensor_tensor(
                    acc[:], y[:, :SC], czh[:, 0:1], acc[:], AL.mult, AL.add
                )

                zt = wk.tile([P, SC], F32, tag="zt")
                nc.scalar.activation(zt[:], acc[:], ACTF.Sigmoid)
                na = wk.tile([P, SC + 2], MBF16 if fp8_z else F32, tag="y", name="na")
                nc.scalar.activation(na[:, :SC], acc[:], ACTF.Sigmoid, scale=-1.0)
                # a = (1-z)*u, in place
                nc.vector.tensor_tensor(
                    na[:, :SC], na[:, :SC], u_sb[:, 2 : SC + 2], AL.mult
                )
                # b = z*th, in place over zt
                for half, ps in zip((0, 1), ps_h):
                    nc.vector.tensor_tensor(
                        zt[:, half * 512 : half * 512 + 512],
                        zt[:, half * 512 : half * 512 + 512],
                        ps[:],
                        AL.mult,
                    )

                hlA = wk.tile([P, 2, SC], MBF16, tag="hlA")
                nc.vector.tensor_tensor_scan(
                    hlA[:, 1, :], na[:, :SC], zt[:], 0.0, AL.mult, AL.add
                )
                # A-scan: u==1 wherever a!=0, so a*u*A == a*A
                nc.vector.tensor_tensor_scan(
                    hlA[:, 0, :], na[:, :SC], u_sb[:, 2 : SC + 2], 1.0,
                    AL.mult, AL.mult,
                )
                nc.scalar.copy(summA[:, m : m + 1], hlA[:, 0, SC - 1 : SC])
                nc.scalar.copy(summH[:, m : m + 1], hlA[:, 1, SC - 1 : SC])
                nc.sync.dma_start(hlA_d[m], hlA[:])
            scopeA.__exit__(None, None, None)

            # ---- phase B (launch): AllGather of scan summaries -----------------
            # Only the summary DMAs + the collective itself go here (gpsimd
            # queue) so nothing downstream head-blocks the sync queue while
            # the collective is in flight; the gather readback + carry chain
            # are emitted after phase A2.
            nc.scalar.dma_start(summ_d[:, 0:64], summA[:])
            nc.scalar.dma_start(summ_d[:, 64:128], summH[:])
            nc.gpsimd.collective_compute(
                "AllGather",
                AL.bypass,
                replica_groups=[list(range(NC))],
                ins=[summ_d.opt()],
                outs=[gath_d.opt()],
            )

            # ---- phase A2: w0 projection + silu, resident in SBUF --------------
            # The carry-chain consumption (gather readback + 16 tiny DVE ops)
            # is emitted mid-loop, once the collective is certainly complete,
            # so phase D starts unblocked the moment A2's matmuls finish.
            state = cpool.tile([P, MT], F32, tag="cstate")
            tmp_c = cpool.tile([P, MT], F32, tag="ctmp")
            mycarry = cpool.tile([P, MT], F32, tag="mycarry")
            scopeC = nc.named_scope("phaseA2"); scopeC.__enter__()
            for m in range(MT):
                w_sb = wpool.tile([P, KT, P], MBF16, tag="wh", name="w_sb")
                nc.sync.dma_start(w_sb[:], w_in[m])
                for half in (0, 1):
                    ps = pp.tile([P, 512], F32, tag="ps")
                    for k in range(KT):
                        nc.tensor.matmul(
                            ps[:],
                            w_sb[:, k],
                            xt_sb[:, k, half * 512 : half * 512 + 512],
                            start=(k == 0),
                            stop=(k == KT - 1),
                        )
                    nc.scalar.activation(
                        silu_sb[:, m, half * 512 : half * 512 + 512],
                        ps[:],
                        ACTF.Silu,
                    )
                if m == 11:
                    # ---- phase B (consume): redundant carry chain --------------
                    nc.any.memset(state[:], 0.0)
                    nc.any.memset(mycarry[:], 0.0)
                    for r in range(NC):
                        gs = wk.tile([P, SC], F32, tag="zt", name=f"gs{r}")
                        nc.sync.dma_start(gs[:, 0:128], gath_d[r])
                        if r > 0:
                            nc.vector.scalar_tensor_tensor(
                                mycarry[:], state[:], sel_sb[:, r : r + 1],
                                mycarry[:], AL.mult, AL.add,
                            )
                        if r < NC - 1:
                            nc.vector.tensor_tensor(
                                tmp_c[:], state[:], gs[:, 0:MT], AL.mult
                            )
                            nc.vector.tensor_tensor(
                                state[:], tmp_c[:], gs[:, 64 : 64 + MT], AL.add
                            )
            scopeC.__exit__(None, None, None)

            # ---- phase D: fused carry fixup + down-projection ------------------
            # Flat (block, m) step list with DMA lookahead: loads for the next
            # block are emitted before the previous block's PSUM drain, so
            # drain waits never head-block the load queues (hlA on sync, wo on
            # scalar).
            scopeD = nc.named_scope("phaseD"); scopeD.__enter__()
            steps = [
                (mb, nbb, m) for mb in (0, 1) for nbb in (0, 1) for m in range(MT)
            ]
            LOOK = 2 if fp8_z else 3
            loaded = {}

            def issue_load(step):
                mb, nbb, m = step
                hlA_rd = dl.tile([P, 2, 512], MBF16, tag="hlard")
                nc.sync.dma_start(
                    hlA_rd[:], hlA_d[m][:, :, mb * 512 : mb * 512 + 512]
                )
                wo_rd = dl.tile([P, 1024], MBF16, tag="word")
                nc.scalar.dma_start(
                    wo_rd[:], wo_in[m][:, nbb * 1024 : nbb * 1024 + 1024]
                )
                loaded[step] = (hlA_rd, wo_rd)

            for j0 in range(LOOK):
                issue_load(steps[j0])
            ps_o = None
            for i, step in enumerate(steps):
                if i + LOOK < len(steps):
                    issue_load(steps[i + LOOK])
                mb, nbb, m = step
                hlA_rd, wo_rd = loaded.pop(step)
                if m == 0:
                    ps_o = [
                        pp.tile([P, 512], F32, tag="ps", name=f"pso{mb}{nbb}{k}")
                        for k in range(8)
                    ]
                t1 = wk.tile([P, 512], MBF16, tag="g", name="t1")
                nc.vector.scalar_tensor_tensor(
                    t1[:], hlA_rd[:, 0, :], mycarry[:, m : m + 1],
                    hlA_rd[:, 1, :], AL.mult, AL.add,
                )
                g = wk.tile([P, 512], MBF16, tag="g")
                nc.vector.tensor_tensor(
                    g[:], t1[:], silu_sb[:, m, mb * 512 : mb * 512 + 512], AL.mult
                )
                for i_m in range(4):
                    for j in range(2):
                        nc.tensor.matmul(
                            ps_o[i_m * 2 + j][:],
                            g[:, i_m * P : (i_m + 1) * P],
                            wo_rd[:, j * 512 : j * 512 + 512],
                            start=(m == 0),
                            stop=(m == MT - 1),
                        )
                if m == MT - 1:
                    # drain on scalar+vector; out DMAs from the scalar queue
                    for i_m in range(4):
                        for j in range(2):
                            o_sb = wk.tile([P, 512], F32, tag="osb")
                            if j == 0:
                                nc.scalar.copy(o_sb[:], ps_o[i_m * 2 + j][:])
                            else:
                                nc.vector.tensor_copy(o_sb[:], ps_o[i_m * 2 + j][:])
                            nc.scalar.dma_start(
                                out_d[
                                    (mb * 4 + i_m) * P : (mb * 4 + i_m + 1) * P,
                                    nbb * 1024 + j * 512 : nbb * 1024
                                    + j * 512
                                    + 512,
                                ],
                                o_sb[:],
                            )
            scopeD.__exit__(None, None, None)
    nc.compile()
    return nc


def _prep_inputs(x, cu_seqlens, w_w, wz_w, wh_w, wo_w, conv_w, NC, fp8_z=FP8_Z):
    """Host-side sharding + layout prep. Returns in_maps list."""
    S, D = x.shape[1], x.shape[2]
    H = w_w.shape[0]
    SC = S // NC
    KT, MT = D // P, H // P
    K2 = KT // 2

    xT = np.ascontiguousarray(x[0].T.astype(np.float32))  # (D, S)

    start = np.zeros(S, np.float32)
    for v in np.asarray(cu_seqlens[:-1]):
        v = int(v)
        if 0 <= v < S:
            start[v] = 1.0
    u = 1.0 - start
    u_full = np.ones(S + 32, np.float32)
    u_full[2 : S + 2] = u  # index j <-> position j-2

    def wprep(wm):  # (H, D) -> (MT, P, KT, P) with [m,p,k,j] = w[m*P+j, k*P+p]
        return np.ascontiguousarray(
            wm.astype(np.float32).reshape(MT, P, KT, P).transpose(0, 3, 2, 1)
        ).astype(BF16)

    wz_f = np.asarray(wz_w, np.float32)
    wh_t, w_t = wprep(wh_w), wprep(w_w)
    if fp8_z:
        wz64 = np.clip(wz_f * WSCALE, -240, 240).astype(FP8)
        wz_t = np.ascontiguousarray(
            wz64.reshape(MT, P, K2, 2, P).transpose(0, 4, 2, 3, 1)
        )
        x16 = np.clip(xT * XSCALE, -240, 240).astype(FP8)
    else:
        wz_t = wprep(wz_w)
    wo_t = np.ascontiguousarray(
        wo_w.T.astype(np.float32).reshape(MT, P, D)
    ).astype(BF16)

    cw_t = conv_w.astype(np.float32)  # (H, CONV)

    in_maps = []
    for c in range(NC):
        s0 = c * SC
        xt_c = np.ascontiguousarray(
            xT[:, s0 : s0 + SC].reshape(KT, P, SC).transpose(1, 0, 2)
        ).astype(BF16)
        # host z_pre history: 3 cols before s0 (zeros at t<0)
        xh = np.zeros((D, 3), np.float32)
        lo = max(0, s0 - 3)
        if s0 > 0:
            xh[:, 3 - (s0 - lo) :] = xT[:, lo:s0]
        zh = wz_f @ xh  # (H, 3)
        czh_c = np.zeros((MT, P, 8), np.float32)
        czh_c[:, :, 0:CONV] = cw_t.reshape(MT, P, CONV)
        czh_c[:, :, 4:7] = zh.reshape(MT, P, 3)
        czh_c = np.ascontiguousarray(czh_c.transpose(1, 0, 2))  # (P, MT, 8)
        u_c = np.ascontiguousarray(
            np.broadcast_to(u_full[s0 : s0 + SC + 32], (P, SC + 32))
        ).astype(BF16)
        sel_c = np.zeros((NC,), np.float32)
        sel_c[c] = 1.0
        sel_c = np.ascontiguousarray(np.broadcast_to(sel_c, (P, NC)))
        imap = {
            "xt": xt_c,
            "wh": wh_t,
            "w": w_t,
            "wo": wo_t,
            "czh": czh_c,
            "u": u_c,
            "sel": sel_c,
        }
        if fp8_z:
            imap["wz8"] = wz_t
            imap["xt8"] = np.ascontiguousarray(
                x16[:, s0 : s0 + SC].reshape(K2, 2, P, SC).transpose(2, 0, 1, 3)
            )
        else:
            imap["wz"] = wz_t
        in_maps.append(imap)
    return in_maps


_NC_CACHE = {}


def run_gru(x, cu_seqlens, w_w, wz_w, wh_w, wo_w, conv_w, NC=8, trace=False):
    S, D = x.shape[1], x.shape[2]
    H = w_w.shape[0]
    SC = S // NC
    key = (D, H, SC, NC, FP8_Z)
    if key not in _NC_CACHE:
        _NC_CACHE[key] = build_gru_kernel(D, H, SC, NC)
    nc = _NC_CACHE[key]
    in_maps = _prep_inputs(x, cu_seqlens, w_w, wz_w, wh_w, wo_w, conv_w, NC)
    res = run_bass_kernel_spmd(nc, in_maps, list(range(NC)), trace=trace)
    out = np.concatenate([res.results[c]["out"] for c in range(NC)], axis=0)
    return out.reshape(1, S, D).astype(np.float32), res


def kernel(**inputs):
    out, _ = run_gru(
        inputs["x"],
        inputs["cu_seqlens"],
        inputs["w_w"],
        inputs["wz_w"],
        inputs["wh_w"],
        inputs["wo_w"],
        inputs["conv_w"],
        NC=8,
    )
    return out


# revision 42
# speedup vs baseline: 1.1549x; 1.0055x over previous
"""Trainium2 Bass kernel for nn_GRU_15461882266204 (minGRU with causal conv gate).

Math (reference):
  w0 = x @ w_w.T ; z0 = x @ wz_w.T ; th = x @ wh_w.T          (S,H)
  z  = sigmoid(causal_conv4(z0, conv_w, segment-masked))
  a  = (1-z) * (1-start) ; b = z * th
  h_t = a_t * h_{t-1} + b_t                                    (scan over S)
  out = (h * silu(w0)) @ wo_w.T                                (S,D)

Strategy: sequence-parallel over 8 NeuronCores (1024 positions each, all 5632
channels per core). Phases:
  A1: per hidden m-tile: z/ht projections (bf16 or fp8-DoubleRow for z),
      causal conv + gates on DVE with host-precomputed boundary masks and
      host-precomputed 3-column z_pre history (kills all narrow matmuls),
      hardware tensor_tensor_scan for h_loc and the cumprod A. h_loc/A go to
      DRAM in bf16 (one merged DMA); chunk-end summaries stay in SBUF.
  B:  360KB AllGather of (A_end, h_end) summaries; every core redundantly
      computes the carry chain. Runs concurrently with A2.
  A2: w0 projection + silu, kept resident in SBUF (bf16).
  D:  fused carry fixup + down-projection: per output block, per m-tile:
      g = (h_loc + A*carry) * silu on DVE feeding bf16 matmuls accumulating
      over all 44 m-tiles in PSUM. Output is sequence-sharded; host concats.
"""
import sys

sys.path.insert(0, "/opt/trn_rl_repo")

import numpy as np

import concourse.bacc as bacc
import concourse.mybir as mybir
import concourse.tile as tile
from concourse.bass_utils import run_bass_kernel_spmd

import ml_dtypes

BF16 = np.dtype(ml_dtypes.bfloat16)
FP8 = np.dtype(ml_dtypes.float8_e4m3)

F32 = mybir.dt.float32
MBF16 = mybir.dt.bfloat16
MFP8 = mybir.dt.float8e4
AL = mybir.AluOpType
ACTF = mybir.ActivationFunctionType
PERF_DR = mybir.MatmulPerfMode.DoubleRow

P = 128
CONV = 4
# fp8 (e4m3, DoubleRow) for the z projection: its error is damped by the
# sigmoid; x scaled by 16, wz by 64 on host, descaled at PSUM drain.
FP8_Z = True
XSCALE = 16.0
WSCALE = 64.0
DESCALE = 1.0 / (XSCALE * WSCALE)


def build_gru_kernel(D, H, SC, NC, fp8_z=FP8_Z):
    KT = D // P          # contraction k-tiles (16)
    K2 = KT // 2         # fp8 DoubleRow k-tiles (8)
    MT = H // P          # hidden m-tiles (44)
    SCH = SC + 3         # z_pre cols incl 3 history cols
    MPT = SC // P        # seq row-tiles (8)

    nc = bacc.Bacc(None, target_bir_lowering=False, debug=False)

    xt_in = nc.declare_dram_parameter("xt", [P, KT, SC], MBF16, isOutput=False)
    wz_in = (
        nc.declare_dram_parameter("wz8", [MT, P, K2, 2, P], MFP8, isOutput=False)
        if fp8_z
        else nc.declare_dram_parameter("wz", [MT, P, KT, P], MBF16, isOutput=False)
    )
    if fp8_z:
        xt8_in = nc.declare_dram_parameter(
            "xt8", [P, K2, 2, SC], MFP8, isOutput=False
        )
    wh_in = nc.declare_dram_parameter("wh", [MT, P, KT, P], MBF16, isOutput=False)
    w_in = nc.declare_dram_parameter("w", [MT, P, KT, P], MBF16, isOutput=False)
    wo_in = nc.declare_dram_parameter("wo", [MT, P, D], MBF16, isOutput=False)
    # czh[:,m]: cols 0-3 conv_w taps, 4-6 host z_pre history, 7 pad
    czh_in = nc.declare_dram_parameter("czh", [P, MT, 8], F32, isOutput=False)
    # padded to 1056 cols: 64B-aligned rows (odd-size DMAs are slow)
    u_in = nc.declare_dram_parameter("u", [P, SC + 32], MBF16, isOutput=False)
    sel_in = nc.declare_dram_parameter("sel", [P, NC], F32, isOutput=False)
    out_d = nc.declare_dram_parameter("out", [SC, D], F32, isOutput=True)

    with tile.TileContext(nc) as tc:
        with (
            tc.tile_pool(name="const", bufs=1) as cpool,
            tc.tile_pool(name="wts", bufs=2) as wpool,
            tc.tile_pool(name="work", bufs=2) as wk,
            tc.tile_pool(name="dload", bufs=4) as dl,
            tc.tile_pool(name="psum", bufs=8, space="PSUM") as pp,
            tc.tile_pool(name="dram", bufs=1, space="DRAM") as dp,
        ):
            # ---- resident tiles ------------------------------------------------
            # xt on the sync queue; u/sel on the scalar (Activation) HWDGE
            # queue so the first m-tile's weight DMAs start right behind xt.
            if fp8_z:
                # z runs first per m-tile: its fp8 activations lead the queue
                xt8_sb = cpool.tile([P, K2, 2, SC], MFP8, tag="xt8")
                nc.scalar.dma_start(xt8_sb[:], xt8_in[:])
            xt_sb = cpool.tile([P, KT, SC], MBF16, tag="xt")
            nc.sync.dma_start(xt_sb[:, :, 0:512], xt_in[:, :, 0:512])
            nc.scalar.dma_start(xt_sb[:, :, 512:SC], xt_in[:, :, 512:SC])
            u_sb = cpool.tile([P, SC + 32], MBF16, tag="u")
            nc.scalar.dma_start(u_sb[:], u_in[:])
            sel_sb = cpool.tile([P, NC], F32, tag="sel")
            nc.scalar.dma_start(sel_sb[:], sel_in[:])
            # all 44 m-tiles' conv taps + z history in ONE dma (tiny rows are
            # descriptor-bound: 44 separate [P,8] DMAs cost ~8us each)
            czh_sb = cpool.tile([P, MT, 8], F32, tag="czh")
            nc.scalar.dma_start(czh_sb[:], czh_in[:])
            summA = cpool.tile([P, 64], F32, tag="summA")
            summH = cpool.tile([P, 64], F32, tag="summH")
            silu_sb = cpool.tile([P, MT, SC], MBF16, tag="silu")

            # internal DRAM bounce buffers
            hlA_d = dp.tile([MT, P, 2, SC], MBF16)  # [:,0,:]=A  [:,1,:]=h_loc
            summ_d = dp.tile([P, 128], F32)
            gath_d = dp.tile([NC, P, 128], F32, addr_space="Shared")

            # ---- phase A1: z/ht matmuls, conv, gating, local scans -------------
            scopeA = nc.named_scope("phaseA1"); scopeA.__enter__()
            for m in range(MT):
                czh = czh_sb[:, m]
                if fp8_z:
                    wz_sb = wpool.tile([P, K2, 2, P], MFP8, tag="wz8")
                else:
                    wz_sb = wpool.tile([P, KT, P], MBF16, tag="wz")
                nc.sync.dma_start(wz_sb[:], wz_in[m])
                wh_sb = wpool.tile([P, KT, P], MBF16, tag="wh")
                nc.sync.dma_start(wh_sb[:], wh_in[m])

                # z_pre: cols 0-2 = host history, cols 3.. = matmul
                zp = wk.tile([P, SCH], MBF16 if fp8_z else F32, tag="zpre")
                nc.scalar.copy(zp[:, 0:3], czh[:, 4:7])
                for half in (0, 1):
                    ps = pp.tile([P, 512], F32, tag="ps")
                    if fp8_z:
                        for k2 in range(K2):
                            nc.tensor.matmul(
                                ps[:],
                                wz_sb[:, k2],
                                xt8_sb[:, k2, :, half * 512 : half * 512 + 512],
                                start=(k2 == 0),
                                stop=(k2 == K2 - 1),
                                perf_mode=PERF_DR,
                            )
                        nc.scalar.activation(
                            zp[:, 3 + half * 512 : 3 + half * 512 + 512],
                            ps[:],
                            ACTF.Copy,
                            scale=DESCALE,
                        )
                    else:
                        for k in range(KT):
                            nc.tensor.matmul(
                                ps[:],
                                wz_sb[:, k],
                                xt_sb[:, k, half * 512 : half * 512 + 512],
                                start=(k == 0),
                                stop=(k == KT - 1),
                            )
                        nc.scalar.copy(
                            zp[:, 3 + half * 512 : 3 + half * 512 + 512], ps[:]
                        )

                ps_h = []
                for half in (0, 1):
                    ps = pp.tile([P, 512], F32, tag="ps")
                    for k in range(KT):
                        nc.tensor.matmul(
                            ps[:],
                            wh_sb[:, k],
                            xt_sb[:, k, half * 512 : half * 512 + 512],
                            start=(k == 0),
                            stop=(k == KT - 1),
                        )
                    ps_h.append(ps)

                # conv: yk(t) = u(t)*y{k-1}(t-1), folded in-place into one tile
                y = wk.tile([P, SC + 2], MBF16 if fp8_z else F32, tag="y")
                nc.vector.tensor_tensor(
                    y[:], u_sb[:, : SC + 2], zp[:, : SC + 2], AL.mult
                )
                acc = wk.tile([P, SC], F32, tag="acc")
                nc.vector.tensor_scalar(
                    acc[:], zp[:, 3:SCH], czh[:, 3:4], None, AL.mult
                )
                nc.vector.scalar_tensor_tensor(
                    acc[:], y[:, 2 : SC + 2], czh[:, 2:3], acc[:], AL.mult, AL.add
                )
                nc.vector.tensor_tensor(
                    y[:, : SC + 1], u_sb[:, 1 : SC + 2], y[:, : SC + 1], AL.mult
                )
                nc.vector.scalar_tensor_tensor(
                    acc[:], y[:, 1 : SC + 1], czh[:, 1:2], acc[:], AL.mult, AL.add
                )
                nc.vector.tensor_tensor(
                    y[:, :SC], u_sb[:, 2 : SC + 2], y[:, :SC], AL.mult
                )
                nc.vector.scalar_tensor_tensor(
                    acc[:], y[:, :SC], czh[:, 0:1], acc[:], AL.mult, AL.add
                )

                zt = wk.tile([P, SC], F32, tag="zt")
                nc.scalar.activation(zt[:], acc[:], ACTF.Sigmoid)
                na = wk.tile([P, SC + 2], MBF16 if fp8_z else F32, tag="y", name="na")
                nc.scalar.activation(na[:, :SC], acc[:], ACTF.Sigmoid, scale=-1.0)
                # a = (1-z)*u, in place
                nc.vector.tensor_tensor(
                    na[:, :SC], na[:, :SC], u_sb[:, 2 : SC + 2], AL.mult
                )
                # b = z*th, in place over zt
                for half, ps in zip((0, 1), ps_h):
                    nc.vector.tensor_tensor(
                        zt[:, half * 512 : half * 512 + 512],
                        zt[:, half * 512 : half * 512 + 512],
                        ps[:],
                        AL.mult,
                    )

                hlA = wk.tile([P, 2, SC], MBF16, tag="hlA")
                nc.vector.tensor_tensor_scan(
                    hlA[:, 1, :], na[:, :SC], zt[:], 0.0, AL.mult, AL.add
                )
                # A-scan: u==1 wherever a!=0, so a*u*A == a*A
                nc.vector.tensor_tensor_scan(
                    hlA[:, 0, :], na[:, :SC], u_sb[:, 2 : SC + 2], 1.0,
                    AL.mult, AL.mult,
                )
                nc.scalar.copy(summA[:, m : m + 1], hlA[:, 0, SC - 1 : SC])
                nc.scalar.copy(summH[:, m : m + 1], hlA[:, 1, SC - 1 : SC])
                nc.sync.dma_start(hlA_d[m], hlA[:])
            scopeA.__exit__(None, None, None)

            # ---- phase B (launch): AllGather of scan summaries -----------------
            # Only the summary DMAs + the collective itself go here (gpsimd
            # queue) so nothing downstream head-blocks the sync queue while
            # the collective is in flight; the gather readback + carry chain
            # are emitted after phase A2.
            nc.scalar.dma_start(summ_d[:, 0:64], summA[:])
            nc.scalar.dma_start(summ_d[:, 64:128], summH[:])
            nc.gpsimd.collective_compute(
                "AllGather",
                AL.bypass,
                replica_groups=[list(range(NC))],
                ins=[summ_d.opt()],
                outs=[gath_d.opt()],
            )

            # ---- phase A2: w0 projection + silu, resident in SBUF --------------
            # The carry-chain consumption (gather readback + 16 tiny DVE ops)
            # is emitted mid-loop, once the collective is certainly complete,
            # so phase D starts unblocked the moment A2's matmuls finish.
            state = cpool.tile([P, MT], F32, tag="cstate")
            tmp_c = cpool.tile([P, MT], F32, tag="ctmp")
            mycarry = cpool.tile([P, MT], F32, tag="mycarry")
            scopeC = nc.named_scope("phaseA2"); scopeC.__enter__()
            for m in range(MT):
                w_sb = wpool.tile([P, KT, P], MBF16, tag="wh", name="w_sb")
                nc.sync.dma_start(w_sb[:], w_in[m])
                for half in (0, 1):
                    ps = pp.tile([P, 512], F32, tag="ps")
                    for k in range(KT):
                        nc.tensor.matmul(
                            ps[:],
                            w_sb[:, k],
                            xt_sb[:, k, half * 512 : half * 512 + 512],
                            start=(k == 0),
                            stop=(k == KT - 1),
                        )
                    nc.scalar.activation(
                        silu_sb[:, m, half * 512 : half * 512 + 512],
                        ps[:],
                        ACTF.Silu,
                    )
                if m == 11:
                    # ---- phase B (consume): redundant carry chain --------------
                    nc.any.memset(state[:], 0.0)
                    nc.any.memset(mycarry[:], 0.0)
                    for r in range(NC):
                        gs = wk.tile([P, SC], F32, tag="zt", name=f"gs{r}")
                        nc.sync.dma_start(gs[:, 0:128], gath_d[r])
                        if r > 0:
                            nc.vector.scalar_tensor_tensor(
                                mycarry[:], state[:], sel_sb[:, r : r + 1],
                                mycarry[:], AL.mult, AL.add,
                            )
                        if r < NC - 1:
                            nc.vector.tensor_tensor(
                                tmp_c[:], state[:], gs[:, 0:MT], AL.mult
                            )
                            nc.vector.tensor_tensor(
                                state[:], tmp_c[:], gs[:, 64 : 64 + MT], AL.add
                            )
            scopeC.__exit__(None, None, None)

            # ---- phase D: fused carry fixup + down-projection ------------------
            # Flat (block, m) step list with DMA lookahead: loads for the next
            # block are emitted before the previous block's PSUM drain, so
            # drain waits never head-block the load queues (hlA on sync, wo on
            # scalar).
            scopeD = nc.named_scope("phaseD"); scopeD.__enter__()
            steps = [
                (mb, nbb, m) for mb in (0, 1) for nbb in (0, 1) for m in range(MT)
            ]
            LOOK = 3
            loaded = {}

            def issue_load(step):
                mb, nbb, m = step
                hlA_rd = dl.tile([P, 2, 512], MBF16, tag="hlard")
                nc.sync.dma_start(
                    hlA_rd[:], hlA_d[m][:, :, mb * 512 : mb * 512 + 512]
                )
                wo_rd = dl.tile([P, 1024], MBF16, tag="word")
                nc.scalar.dma_start(
                    wo_rd[:], wo_in[m][:, nbb * 1024 : nbb * 1024 + 1024]
                )
                loaded[step] = (hlA_rd, wo_rd)

            for j0 in range(LOOK):
                issue_load(steps[j0])
            ps_o = None
            for i, step in enumerate(steps):
                if i + LOOK < len(steps):
                    issue_load(steps[i + LOOK])
                mb, nbb, m = step
                hlA_rd, wo_rd = loaded.pop(step)
                if m == 0:
                    ps_o = [
                        pp.tile([P, 512], F32, tag="ps", name=f"pso{mb}{nbb}{k}")
                        for k in range(8)
                    ]
                t1 = wk.tile([P, 512], MBF16, tag="g", name="t1")
                nc.vector.scalar_tensor_tensor(
                    t1[:], hlA_rd[:, 0, :], mycarry[:, m : m + 1],
                    hlA_rd[:, 1, :], AL.mult, AL.add,
                )
                g = wk.tile([P, 512], MBF16, tag="g")
                nc.vector.tensor_tensor(
                    g[:], t1[:], silu_sb[:, m, mb * 512 : mb * 512 + 512], AL.mult
                )
                for i_m in range(4):
                    for j in range(2):
                        nc.tensor.matmul(
                            ps_o[i_m * 2 + j][:],
                            g[:, i_m * P : (i_m + 1) * P],
                            wo_rd[:, j * 512 : j * 512 + 512],
                            start=(m == 0),
                            stop=(m == MT - 1),
                        )
                if m == MT - 1:
                    # drain on scalar+vector; out DMAs from the scalar queue
                    for i_m in range(4):
                        for j in range(2):
                            o_sb = wk.tile([P, 512], F32, tag="osb")
                            if j == 0:
                                nc.scalar.copy(o_sb[:], ps_o[i_m * 2 + j][:])
                            else:
                                nc.vector.tensor_copy(o_sb[:], ps_o[i_m * 2 + j][:])
                            nc.scalar.dma_start(
                                out_d[
                                    (mb * 4 + i_m) * P : (mb * 4 + i_m + 1) * P,
                                    nbb * 1024 + j * 512 : nbb * 1024
                                    + j * 512
                                    + 512,
                                ],
                                o_sb[:],
                            )
            scopeD.__exit__(None, None, None)
    nc.compile()
    return nc


def _prep_inputs(x, cu_seqlens, w_w, wz_w, wh_w, wo_w, conv_w, NC, fp8_z=FP8_Z):
    """Host-side sharding + layout prep. Returns in_maps list."""
    S, D = x.shape[1], x.shape[2]
    H = w_w.shape[0]
    SC = S // NC
    KT, MT = D // P, H // P
    K2 = KT // 2

    xT = np.ascontiguousarray(x[0].T.astype(np.float32))  # (D, S)

    start = np.zeros(S, np.float32)
    for v in np.asarray(cu_seqlens[:-1]):
        v = int(v)
        if 0 <= v < S:
            start[v] = 1.0
    u = 1.0 - start
    u_full = np.ones(S + 32, np.float32)
    u_full[2 : S + 2] = u  # index j <-> position j-2

    def wprep(wm):  # (H, D) -> (MT, P, KT, P) with [m,p,k,j] = w[m*P+j, k*P+p]
        return np.ascontiguousarray(
            wm.astype(np.float32).reshape(MT, P, KT, P).transpose(0, 3, 2, 1)
        ).astype(BF16)

    wz_f = np.asarray(wz_w, np.float32)
    wh_t, w_t = wprep(wh_w), wprep(w_w)
    if fp8_z:
        wz64 = np.clip(wz_f * WSCALE, -240, 240).astype(FP8)
        wz_t = np.ascontiguousarray(
            wz64.reshape(MT, P, K2, 2, P).transpose(0, 4, 2, 3, 1)
        )
        x16 = np.clip(xT * XSCALE, -240, 240).astype(FP8)
    else:
        wz_t = wprep(wz_w)
    wo_t = np.ascontiguousarray(
        wo_w.T.astype(np.float32).reshape(MT, P, D)
    ).astype(BF16)

    cw_t = conv_w.astype(np.float32)  # (H, CONV)

    in_maps = []
    for c in range(NC):
        s0 = c * SC
        xt_c = np.ascontiguousarray(
            xT[:, s0 : s0 + SC].reshape(KT, P, SC).transpose(1, 0, 2)
        ).astype(BF16)
        # host z_pre history: 3 cols before s0 (zeros at t<0)
        xh = np.zeros((D, 3), np.float32)
        lo = max(0, s0 - 3)
        if s0 > 0:
            xh[:, 3 - (s0 - lo) :] = xT[:, lo:s0]
        zh = wz_f @ xh  # (H, 3)
        czh_c = np.zeros((MT, P, 8), np.float32)
        czh_c[:, :, 0:CONV] = cw_t.reshape(MT, P, CONV)
        czh_c[:, :, 4:7] = zh.reshape(MT, P, 3)
        czh_c = np.ascontiguousarray(czh_c.transpose(1, 0, 2))  # (P, MT, 8)
        u_c = np.ascontiguousarray(
            np.broadcast_to(u_full[s0 : s0 + SC + 32], (P, SC + 32))
        ).astype(BF16)
        sel_c = np.zeros((NC,), np.float32)
        sel_c[c] = 1.0
        sel_c = np.ascontiguousarray(np.broadcast_to(sel_c, (P, NC)))
        imap = {
            "xt": xt_c,
            "wh": wh_t,
            "w": w_t,
            "wo": wo_t,
            "czh": czh_c,
            "u": u_c,
            "sel": sel_c,
        }
        if fp8_z:
            imap["wz8"] = wz_t
            imap["xt8"] = np.ascontiguousarray(
                x16[:, s0 : s0 + SC].reshape(K2, 2, P, SC).transpose(2, 0, 1, 3)
            )
        else:
            imap["wz"] = wz_t
        in_maps.append(imap)
    return in_maps


_NC_CACHE = {}


def run_gru(x, cu_seqlens, w_w, wz_w, wh_w, wo_w, conv_w, NC=8, trace=False):
    S, D = x.shape[1], x.shape[2]
    H = w_w.shape[0]
    SC = S // NC
    key = (D, H, SC, NC, FP8_Z)
    if key not in _NC_CACHE:
        _NC_CACHE[key] = build_gru_kernel(D, H, SC, NC)
    nc = _NC_CACHE[key]
    in_maps = _prep_inputs(x, cu_seqlens, w_w, wz_w, wh_w, wo_w, conv_w, NC)
    res = run_bass_kernel_spmd(nc, in_maps, list(range(NC)), trace=trace)
    out = np.concatenate([res.results[c]["out"] for c in range(NC)], axis=0)
    return out.reshape(1, S, D).astype(np.float32), res


def kernel(**inputs):
    out, _ = run_gru(
        inputs["x"],
        inputs["cu_seqlens"],
        inputs["w_w"],
        inputs["wz_w"],
        inputs["wh_w"],
        inputs["wo_w"],
        inputs["conv_w"],
        NC=8,
    )
    return out
